# revision 1
# baseline (speedup 1.0000x reference)
"""Gemma3 sliding-window attention layer on 8 Trainium2 NeuronCores.

Sharding: tensor-parallel over heads. Core c computes q-head c and kv-head c//2
(kv heads duplicated across the 2 cores sharing them), then the o_proj
row-slice for its head. The 8 partial outputs are summed on the host.

v3 (vs v2):
- startup: per-kt weight/hT DMA tiles, kt-major matmul emission, deferred
  phase-B constants -> first matmul at ~2us instead of ~42us.
- attention corner trim: the two half-masked edge k-tiles compute only the
  valid 128-query half (full-width tiles open each PSUM accumulation group).
- PSUM pools: ssq+rb merged, sums+rbv merged -> xp bufs 5, sc bufs 3.
"""
import os
import sys
import types
import contextlib
import ctypes

import numpy as np
import ml_dtypes

for _p in ("/opt/trn_rl_repo", "/root/.axon_site/_ro/trn_rl_repo"):
    if os.path.isdir(_p) and _p not in sys.path:
        sys.path.insert(0, _p)

from contextlib import ExitStack

import concourse.bass as bass
import concourse.mybir as mybir
import concourse.tile as tile
from concourse import bacc
from concourse.bass_utils import run_bass_kernel_spmd

S = 4096
HID = 2560
NH = 8
NKV = 4
HD = 256
WIN = 1024
ROPE_BASE = 10000.0
EPS = 1e-6
SCALING = HD ** -0.5

NCORES = 8
CA = 512            # tokens per projection chunk (phase A)
NCA = S // CA       # 8
CB = 256            # queries per attention block (phase B)
NCB = S // CB       # 16
KT = HID // 128     # 20 hid k-tiles
f32 = mybir.dt.float32
f32r = mybir.dt.float32r
bf16 = mybir.dt.bfloat16
AF = mybir.ActivationFunctionType
BF = ml_dtypes.bfloat16

_NC = None
_last_results = None


def _install_ntff_shim():
    """antenv.axon_hooks is absent in this image; rebuild it over libaxon so
    run_bass_kernel_spmd(trace=True) can capture NTFF profiles."""
    if "antenv.axon_hooks" in sys.modules:
        return
    so_path = "/opt/axon/libaxon_pjrt.so"
    hook = None
    try:
        lib = ctypes.CDLL(so_path)
        if hasattr(lib, "axon_start_nrt_profile"):
            lib.axon_start_nrt_profile.argtypes = [
                ctypes.POINTER(ctypes.c_int64),
                ctypes.c_size_t,
            ]
            lib.axon_start_nrt_profile.restype = ctypes.c_int64
            lib.axon_stop_nrt_profile.argtypes = [ctypes.c_char_p]
            lib.axon_stop_nrt_profile.restype = ctypes.c_int64

            @contextlib.contextmanager
            def _hook(output_dir, device_ids):
                import jax

                jax.devices()
                if device_ids:
                    ids = (ctypes.c_int64 * len(device_ids))(*device_ids)
                    rc = lib.axon_start_nrt_profile(ids, len(device_ids))
                else:
                    rc = lib.axon_start_nrt_profile(None, 0)
                if rc != 0:
                    raise RuntimeError(f"axon_start_nrt_profile rc={rc}")
                try:
                    yield
                finally:
                    n = lib.axon_stop_nrt_profile(str(output_dir).encode())
                    if n < 0:
                        raise RuntimeError(f"axon_stop_nrt_profile rc={n}")

            hook = _hook
    except OSError:
        pass
    mod = types.ModuleType("antenv.axon_hooks")
    mod.get_axon_ntff_profile_hook = lambda: hook
    mod.set_axon_ntff_profile_hook = lambda h: None
    sys.modules["antenv.axon_hooks"] = mod


def _phase_a(tc, nc, hT, hTkv, w, cs, cskv, ow, msk, cs_sb, cskv_sb, ow_sb,
             msk_sb, inw_sb, on_sb, on1_sb, qT, kv, vt, qkvres, const):
    """A1: k+v projection for this core's HALF of the sequence (the pair
    partner computes the other half), results packed to a DRAM bounce and
    pair-AllGathered. A2: q projection for the full sequence (overlaps the
    collective). Unpack DMAs then fill the kv/v tiles for phase B."""
    NKC = NCA // 2  # kv chunks computed locally

    def norm_rope(small, rtmp, nrmp, wo, dest, xps, cos, sin):
        x0p, x1p = xps
        sq0 = sqpool.tile([128, CA], bf16, tag="sq")
        sq1 = sqpool.tile([128, CA], bf16, tag="sq")
        nc.scalar.activation(sq0, x0p, AF.Square, bias=0.0,
                             scale=inw_sb[:, wo:wo + 1])
        nc.scalar.activation(sq1, x1p, AF.Square, bias=0.0,
                             scale=inw_sb[:, wo + 1:wo + 2])
        ssq = nrmp.tile([1, CA], f32, tag="nrm")
        nc.tensor.matmul(ssq, on_sb[:, 0:1], sq0, start=True, stop=False)
        nc.tensor.matmul(ssq, on_sb[:, 0:1], sq1, start=False, stop=True)
        t1 = small.tile([1, CA], f32, tag="t1")
        nc.scalar.activation(t1, ssq, AF.Copy, bias=EPS, scale=1.0 / HD)
        t2 = small.tile([1, CA], f32, tag="t2")
        nc.vector.reciprocal_approx_fast(out=t2, in_=t1)
        rstd = small.tile([1, CA], f32r, tag="rstd")
        nc.scalar.activation(rstd, t2, AF.Sqrt, bias=0.0, scale=1.0)
        rb = nrmp.tile([128, CA], f32, tag="nrm")
        nc.tensor.matmul(rb, on1_sb, rstd, start=True, stop=True)
        ra = rtmp.tile([128, CA], f32, tag="m")
        rb_ = rtmp.tile([128, CA], f32, tag="m")
        nc.vector.tensor_mul(ra, x0p, cos)
        nc.vector.tensor_mul(rb_, x1p, sin)
        re = rtmp.tile([128, CA], f32, tag="m")
        nc.vector.tensor_sub(re, ra, rb_)
        rc = rtmp.tile([128, CA], f32, tag="m")
        rd = rtmp.tile([128, CA], f32, tag="m")
        nc.vector.tensor_mul(rc, x1p, cos)
        nc.vector.tensor_mul(rd, x0p, sin)
        rf = rtmp.tile([128, CA], f32, tag="m")
        nc.vector.tensor_add(rf, rc, rd)
        nc.vector.tensor_mul(dest[:, 0:CA], re, rb)
        nc.vector.tensor_mul(dest[:, CA:2 * CA], rf, rb)

    with tc.tile_pool(name="hTt", bufs=2) as hpool, \
         tc.tile_pool(name="hT0", bufs=1) as h0pool, \
         tc.tile_pool(name="kvloc", bufs=1) as locpool, \
         tc.tile_pool(name="sq", bufs=2) as sqpool, \
         tc.tile_pool(name="rtmpA", bufs=4) as rtmp, \
         tc.tile_pool(name="smallA", bufs=1) as small, \
         tc.tile_pool(name="dramx", bufs=1, space="DRAM") as dram, \
         tc.tile_pool(name="xp", bufs=5, space="PSUM") as xpp, \
         tc.tile_pool(name="vps", bufs=2, space="PSUM") as vpp, \
         tc.tile_pool(name="nrm", bufs=1, space="PSUM") as nrmp:

        bounce_in = [dram.tile([128, 2048], bf16, name="bin0"),
                     dram.tile([128, 6144], bf16, name="bin1")]
        bounce_out = [dram.tile([256, 2048], bf16, name="bout0"),
                      dram.tile([256, 6144], bf16, name="bout1")]

        # interleaved per-kt startup DMAs: kv-half chunk-0 hT slice then the
        # kt's weights, so the first matmuls unblock after ~2 slices.
        w_kt = []
        h0_kt = []
        for k in range(KT):
            h0 = h0pool.tile([128, CA], bf16, tag=f"h0_{k}")
            nc.sync.dma_start(out=h0, in_=hTkv[:, k * CA:(k + 1) * CA])
            h0_kt.append(h0)
            wt = const.tile([128, 768], bf16, tag=f"w_{k}")
            nc.sync.dma_start(out=wt, in_=w[:, k * 768:(k + 1) * 768])
            w_kt.append(wt)
        nc.gpsimd.dma_start(out=cskv_sb, in_=cskv)

        # ---- A1: k+v for the local half-sequence ----
        for a in range(NKC):
            if a == 0:
                hslc = h0_kt
            else:
                hTt = hpool.tile([128, KT * CA], bf16, tag="hTt")
                nc.sync.dma_start(out=hTt,
                                  in_=hTkv[:, a * KT * CA:(a + 1) * KT * CA])
                hslc = [hTt[:, k * CA:(k + 1) * CA] for k in range(KT)]
            cos = cskv_sb[:, a * 2 * CA: a * 2 * CA + CA]
            sin = cskv_sb[:, a * 2 * CA + CA: (a + 1) * 2 * CA]

            kvt = locpool.tile([128, 2 * CA], bf16, tag="kvloc", bufs=2, name=f"kvloc{a}")
            x_k = [xpp.tile([128, CA], f32, tag="xp", name=f"xk{a}_{j}")
                   for j in range(2)]
            vps = [vpp.tile([128, HD], f32, tag="vps", name=f"vp{a}_{j}")
                   for j in range(CA // 128)]
            for k in range(KT):
                st_, sp = (k == 0), (k == KT - 1)
                for j in range(2):
                    nc.tensor.matmul(
                        x_k[j], w_kt[k][:, 256 + j * 128:256 + (j + 1) * 128],
                        hslc[k], start=st_, stop=sp)
                for st in range(CA // 128):
                    nc.tensor.matmul(
                        vps[st], hslc[k][:, st * 128:(st + 1) * 128],
                        w_kt[k][:, 512:768], start=st_, stop=sp)

            norm_rope(small, rtmp, nrmp, 2, kvt, x_k, cos, sin)
            ci = 0 if a == 0 else 1
            slot = 0 if a == 0 else a - 1
            base = slot * 2048
            nc.gpsimd.dma_start(
                out=bounce_in[ci][:, base:base + 1024], in_=kvt)
            for st in range(CA // 128):
                vtile = locpool.tile([128, HD], bf16, tag="vloc", bufs=5, name=f"vloc{a}_{st}")
                nc.scalar.activation(vtile, vps[st], AF.Copy, bias=0.0,
                                     scale=1.0)
                nc.gpsimd.dma_start(
                    out=bounce_in[ci][:, base + 1024 + st * HD:
                                      base + 1024 + (st + 1) * HD],
                    in_=vtile)
            if a == 0 or a == NKC - 1:
                nc.gpsimd.collective_compute(
                    "AllGather",
                    mybir.AluOpType.bypass,
                    replica_groups=[[0, 1], [2, 3], [4, 5], [6, 7]],
                    ins=[bounce_in[ci].opt()],
                    outs=[bounce_out[ci].opt()],
                )
        # bulky later-phase constants: dispatched from the DVE queue after
        # the A1 rope work, so their transfers don't starve the startup
        # slices but still land before A2 rope / phase B need them.
        nc.scalar.dma_start(out=cs_sb, in_=cs)
        nc.scalar.dma_start(out=ow_sb, in_=ow)
        nc.scalar.dma_start(out=msk_sb, in_=msk)

        # ---- A2: q for the full sequence (overlaps the collective) ----
        for a in range(NCA):
            hTt = hpool.tile([128, KT * CA], bf16, tag="hTt")
            nc.sync.dma_start(out=hTt,
                              in_=hT[:, a * KT * CA:(a + 1) * KT * CA])
            hslc = [hTt[:, k * CA:(k + 1) * CA] for k in range(KT)]
            cos = cs_sb[:, a * 2 * CA: a * 2 * CA + CA]
            sin = cs_sb[:, a * 2 * CA + CA: (a + 1) * 2 * CA]

            qTt = qkvres.tile([128, 2 * CA], bf16, tag=f"qT{a}")
            qT[a] = qTt
            x_q = [xpp.tile([128, CA], f32, tag="xp", name=f"xq{a}_{j}")
                   for j in range(2)]
            for k in range(KT):
                st_, sp = (k == 0), (k == KT - 1)
                for j in range(2):
                    nc.tensor.matmul(
                        x_q[j], w_kt[k][:, j * 128:(j + 1) * 128],
                        hslc[k], start=st_, stop=sp)
            norm_rope(small, rtmp, nrmp, 0, qTt, x_q, cos, sin)

        # ---- unpack the gathered kv/v for the full sequence ----
        for a in range(NCA):
            mrow = (a // NKC) * 128
            la = a % NKC
            ci = 0 if la == 0 else 1
            base = (0 if la == 0 else (la - 1)) * 2048
            kvt = qkvres.tile([128, 2 * CA], bf16, tag=f"kv{a}")
            nc.scalar.dma_start(
                out=kvt,
                in_=bounce_out[ci][mrow:mrow + 128, base:base + 1024])
            kv[a] = kvt
            for st in range(CA // 128):
                vtile = qkvres.tile([128, HD], bf16, tag=f"v{a}_{st}")
                nc.scalar.dma_start(
                    out=vtile,
                    in_=bounce_out[ci][mrow:mrow + 128,
                                       base + 1024 + st * HD:
                                       base + 1024 + (st + 1) * HD])
                vt[(CA // 128) * a + st] = vtile


def _phase_b(tc, nc, ow_sb, msk_sb, on_sb, on1_sb, qT, kv, vt, outp):
    with tc.tile_pool(name="probs", bufs=12) as ppool, \
         tc.tile_pool(name="attnT", bufs=4) as apool, \
         tc.tile_pool(name="osb", bufs=2) as opool, \
         tc.tile_pool(name="ibsp", bufs=2) as ipool, \
         tc.tile_pool(name="smallB", bufs=2) as small, \
         tc.tile_pool(name="sc", bufs=3, space="PSUM") as scp, \
         tc.tile_pool(name="pv", bufs=2, space="PSUM") as pvp, \
         tc.tile_pool(name="sums", bufs=1, space="PSUM") as smp, \
         tc.tile_pool(name="op", bufs=2, space="PSUM") as opp:
        for t in range(NCB):
            a, half = t // 2, t % 2
            t0 = t * CB
            qs = qT[a]

            # k-subtiles, full-width ones first (they open the accumulation
            # groups); the two half-masked edges compute only the valid
            # 128-query half.
            #   (kt, qoff, width, mask)
            plan = []
            for kt in range(max(0, 2 * t - 7), 2 * t):
                # kt == 2t-7 is the edge tile of the SECOND query half
                m = ("edge", 128) if kt == 2 * t - 7 else None
                plan.append((kt, 0, CB, m))
            plan.append((2 * t, 0, CB, ("diag", 0)))
            if 2 * t - 8 >= 0:
                plan.append((2 * t - 8, 0, 128, ("edge", 0)))
            plan.append((2 * t + 1, 128, 128, ("diag", 128)))

            prs = []
            for kt, qoff, width, maskspec in plan:
                ca, sb = kt // 4, kt % 4
                kvsrc = kv[ca]
                sc = scp.tile([128, CB], f32, tag="sc")
                scv = sc[:, qoff:qoff + width]
                for h in range(2):
                    nc.tensor.matmul(
                        scv,
                        kvsrc[:, h * CA + sb * 128: h * CA + sb * 128 + 128],
                        qs[:, h * CA + half * CB + qoff:
                           h * CA + half * CB + qoff + width],
                        start=(h == 0), stop=(h == 1))
                pr = ppool.tile([128, CB], bf16, tag="pr")
                prv = pr[:, qoff:qoff + width]
                nc.scalar.activation(prv, scv, AF.Exp, bias=0.0,
                                     scale=SCALING)
                if maskspec is not None:
                    kind, moff = maskspec
                    m = msk_sb[:, 0:128] if kind == "edge" \
                        else msk_sb[:, 256:384]
                    nc.vector.tensor_mul(pr[:, moff:moff + 128],
                                         pr[:, moff:moff + 128], m)
                prs.append(prv)

            sums = smp.tile([1, CB], f32, tag="sums")
            for i, ((kt, qoff, width, _), prv) in enumerate(zip(plan, prs)):
                nc.tensor.matmul(sums[:, qoff:qoff + width], on_sb[:, 0:1],
                                 prv, start=(i == 0), stop=(i == len(prs) - 1))
            pv0 = pvp.tile([128, CB], f32, tag="pv")
            pv1 = pvp.tile([128, CB], f32, tag="pv")
            for i, ((kt, qoff, width, _), prv) in enumerate(zip(plan, prs)):
                first, last = (i == 0), (i == len(plan) - 1)
                v_ = vt[kt]
                nc.tensor.matmul(pv0[:, qoff:qoff + width], v_[:, 0:128], prv,
                                 start=first, stop=last)
                nc.tensor.matmul(pv1[:, qoff:qoff + width], v_[:, 128:256],
                                 prv, start=first, stop=last)

            sc_ = small.tile([1, CB], f32r, tag="sc_")
            nc.scalar.activation(sc_, sums, AF.Copy, bias=0.0, scale=1.0)
            rbv = smp.tile([128, CB], f32, tag="sums")
            nc.tensor.matmul(rbv, on1_sb, sc_, start=True, stop=True)
            ibs = ipool.tile([128, CB], f32, tag="ibs")
            nc.vector.reciprocal_approx_fast(out=ibs, in_=rbv)
            at0 = apool.tile([128, CB], bf16, tag="at")
            at1 = apool.tile([128, CB], bf16, tag="at")
            nc.vector.tensor_mul(at0, pv0, ibs)
            nc.vector.tensor_mul(at1, pv1, ibs)

            # o_proj row-slice: partial [256 tok, HID]
            for st in range(2):
                ob = opool.tile([128, HID], bf16, tag="ob")
                for hc in range(HID // 512):
                    op = opp.tile([128, 512], f32, tag="op")
                    nc.tensor.matmul(op, at0[:, st * 128:(st + 1) * 128],
                                     ow_sb[:, hc * 512:(hc + 1) * 512],
                                     start=True, stop=False)
                    nc.tensor.matmul(op, at1[:, st * 128:(st + 1) * 128],
                                     ow_sb[:, HID + hc * 512:HID + (hc + 1) * 512],
                                     start=False, stop=True)
                    if hc < 3:
                        nc.vector.tensor_copy(ob[:, hc * 512:(hc + 1) * 512],
                                              op)
                    else:
                        nc.scalar.activation(ob[:, hc * 512:(hc + 1) * 512],
                                             op, AF.Copy, bias=0.0, scale=1.0)
                nc.sync.dma_start(
                    out=outp[t0 + st * 128:t0 + (st + 1) * 128, :], in_=ob)


def _body(ctx, tc, hT, hTkv, w, ow, cs, cskv, msk, inw, on, on1, outp):
    nc = tc.nc

    const = ctx.enter_context(tc.tile_pool(name="const", bufs=1))
    qkvres = ctx.enter_context(tc.tile_pool(name="qkvres", bufs=1))

    # small constants first (cheap), then phase A drives its own per-kt DMAs;
    # bulky phase-B constants (ow/msk) and cos/sin stream during phase A.
    inw_sb = const.tile([128, 4], f32)
    nc.sync.dma_start(out=inw_sb, in_=inw)
    on_sb = const.tile([128, 2], bf16)
    nc.sync.dma_start(out=on_sb, in_=on)
    on1_sb = const.tile([1, 128], f32r)
    nc.sync.dma_start(out=on1_sb, in_=on1)
    cs_sb = const.tile([128, NCA * 2 * CA], bf16)
    cskv_sb = const.tile([128, NCA * CA], bf16)
    ow_sb = const.tile([128, 2 * HID], bf16)
    msk_sb = const.tile([128, 384], bf16)

    qT = {}
    kv = {}
    vt = {}

    _phase_a(tc, nc, hT, hTkv, w, cs, cskv, ow, msk, cs_sb, cskv_sb, ow_sb,
             msk_sb, inw_sb, on_sb, on1_sb, qT, kv, vt, qkvres, const)
    _phase_b(tc, nc, ow_sb, msk_sb, on_sb, on1_sb, qT, kv, vt, outp)


def _build():
    nc = bacc.Bacc("TRN2", target_bir_lowering=False, debug=False,
                   num_devices=NCORES)
    hT = nc.dram_tensor("hT", [128, KT * S], bf16, kind="ExternalInput").ap()
    hTkv = nc.dram_tensor("hTkv", [128, KT * S // 2], bf16,
                          kind="ExternalInput").ap()
    cskv = nc.dram_tensor("cskv", [128, NCA * CA], bf16,
                          kind="ExternalInput").ap()
    w = nc.dram_tensor("w", [128, KT * 768], bf16, kind="ExternalInput").ap()
    ow = nc.dram_tensor("ow", [128, 2 * HID], bf16, kind="ExternalInput").ap()
    cs = nc.dram_tensor("cs", [128, NCA * 2 * CA], bf16, kind="ExternalInput").ap()
    msk = nc.dram_tensor("msk", [128, 384], bf16, kind="ExternalInput").ap()
    inw = nc.dram_tensor("inw", [128, 4], f32, kind="ExternalInput").ap()
    on = nc.dram_tensor("on", [128, 2], bf16, kind="ExternalInput").ap()
    on1 = nc.dram_tensor("on1", [1, 128], f32r, kind="ExternalInput").ap()
    outp = nc.dram_tensor("outp", [S, HID], bf16, kind="ExternalOutput").ap()
    with tile.TileContext(nc) as tc, ExitStack() as ctx:
        with nc.allow_low_precision(reason="bf16 matmul pipeline"):
            _body(ctx, tc, hT, hTkv, w, ow, cs, cskv, msk, inw, on, on1, outp)
    nc.compile()
    return nc


def _get_nc():
    global _NC
    if _NC is None:
        _NC = _build()
    return _NC


def build_in_maps(positions, hidden_states, qkv_w, o_w, q_norm_w, k_norm_w):
    positions = np.asarray(positions)
    hidden_states = np.asarray(hidden_states, dtype=np.float32)
    qkv_w = np.asarray(qkv_w, dtype=np.float32)
    o_w = np.asarray(o_w, dtype=np.float32)
    q_norm_w = np.asarray(q_norm_w, dtype=np.float32)
    k_norm_w = np.asarray(k_norm_w, dtype=np.float32)
    assert np.array_equal(positions.astype(np.int64), np.arange(S)), \
        "kernel assumes contiguous arange positions (banded sliding window)"

    hT0 = hidden_states.T  # [HID, S]
    hT = np.ascontiguousarray(
        hT0.reshape(KT, 128, NCA, CA).transpose(1, 2, 0, 3)
        .reshape(128, KT * S)).astype(BF)

    inv_freq = 1.0 / (ROPE_BASE ** (np.arange(0, HD, 2, dtype=np.float32) / HD))
    freqs = positions.astype(np.float32)[:, None] * inv_freq[None, :]  # [S,128]
    cos_t = np.ascontiguousarray(np.cos(freqs).T.astype(np.float32))
    sin_t = np.ascontiguousarray(np.sin(freqs).T.astype(np.float32))
    csb = np.stack([cos_t.reshape(128, NCA, CA), sin_t.reshape(128, NCA, CA)],
                   axis=2)  # [128, NCA, 2, CA]
    cs = np.ascontiguousarray(csb.reshape(128, NCA * 2 * CA)).astype(BF)

    kl = np.arange(128)[:, None]
    ql = np.arange(128)[None, :]
    edge = (kl > ql).astype(np.float32)
    diag = (kl <= ql).astype(np.float32)
    zero = np.zeros((128, 128), np.float32)
    msk = np.concatenate([edge, zero, diag], axis=1).astype(BF)  # [128, 384]

    nwq = 1.0 + q_norm_w
    nwk = 1.0 + k_norm_w
    inw = np.stack([1.0 / nwq[:128], 1.0 / nwq[128:],
                    1.0 / nwk[:128], 1.0 / nwk[128:]], axis=1)
    inw = np.ascontiguousarray(inw.astype(np.float32))  # [128, 4]

    on = np.ones((128, 2), BF)
    on1 = np.ones((1, 128), np.float32)

    in_maps = []
    for c in range(NCORES):
        g = c // 2
        wq = qkv_w[:, c * HD:(c + 1) * HD] * nwq[None, :]
        wk = qkv_w[:, NH * HD + g * HD:NH * HD + (g + 1) * HD] * nwk[None, :]
        wv = qkv_w[:, (NH + NKV) * HD + g * HD:(NH + NKV) * HD + (g + 1) * HD]
        wslice = np.concatenate([wq, wk, wv], axis=1).astype(np.float32)
        wslice = np.ascontiguousarray(
            wslice.reshape(KT, 128, 768).transpose(1, 0, 2)
            .reshape(128, KT * 768)).astype(BF)
        owslice = o_w[c * HD:(c + 1) * HD, :].astype(np.float32)
        owslice = np.ascontiguousarray(
            owslice.reshape(2, 128, HID).transpose(1, 0, 2)
            .reshape(128, 2 * HID)).astype(BF)
        halfsz = KT * S // 2
        hTkv = np.ascontiguousarray(hT[:, (c % 2) * halfsz:
                                        (c % 2 + 1) * halfsz])
        cskv = np.ascontiguousarray(cs[:, (c % 2) * (NCA * CA):
                                       (c % 2 + 1) * (NCA * CA)])
        in_maps.append({
            "hT": hT, "hTkv": hTkv, "cskv": cskv, "w": wslice, "ow": owslice,
            "cs": cs, "msk": msk, "inw": inw, "on": on, "on1": on1,
        })
    return in_maps


def kernel(positions, hidden_states, qkv_w, o_w, q_norm_w, k_norm_w):
    global _last_results
    _install_ntff_shim()
    in_maps = build_in_maps(positions, hidden_states, qkv_w, o_w,
                            q_norm_w, k_norm_w)

    nc = _get_nc()
    res = run_bass_kernel_spmd(nc, in_maps, list(range(NCORES)))
    _last_results = res

    out = res.results[0]["outp"].astype(np.float32)
    for c in range(1, NCORES):
        out = out + res.results[c]["outp"].astype(np.float32)
    return out



# revision 6
# speedup vs baseline: 1.0513x; 1.0513x over previous
"""Gemma3 sliding-window attention layer on 8 Trainium2 NeuronCores.

Sharding: tensor-parallel over heads. Core c computes q-head c and kv-head c//2
(kv heads duplicated across the 2 cores sharing them), then the o_proj
row-slice for its head. The 8 partial outputs are summed on the host.

v4 (vs v3):
- startup: 3-way split contiguous w/h DMAs (big transfers at high BW)
  instead of 40 small per-kt DMAs -> first matmul ~5us instead of ~24us.
- per-chunk pair-AllGather (4 small collectives issued as each local kv
  chunk completes) instead of 2 late ones -> collective fully hidden
  under the q-projection pass; unpack DMAs ride the gpsimd queue right
  after each collective so phase B never waits.
- shared 3-buf h pool gates A2 prefetch behind A1 compute (no early
  bandwidth steal), single [128,10240] DMA per chunk.
- phase B software pipeline: o_proj of block t-1 emitted after the
  attention matmuls of block t, so the softmax-denominator chain
  (sums -> sc_ -> rbv -> ibs -> at) never stalls the PE.
- PSUM pools rebalanced: A = xp4+vps2+ssq1+rb1, B = sc3+pv2+sums1+op2
  (pv0|pv1 packed per-bank; rbv allocated from the sc pool).
- v tiles packed per chunk [128, 4*HD]; 2 bounce writes + 4 unpacks per
  chunk instead of 5/5.
"""
import os
import sys
import types
import contextlib
import ctypes

import numpy as np
import ml_dtypes

for _p in ("/opt/trn_rl_repo", "/root/.axon_site/_ro/trn_rl_repo"):
    if os.path.isdir(_p) and _p not in sys.path:
        sys.path.insert(0, _p)

from contextlib import ExitStack

import concourse.bass as bass
import concourse.mybir as mybir
import concourse.tile as tile
from concourse import bacc
from concourse.bass_utils import run_bass_kernel_spmd

S = 4096
HID = 2560
NH = 8
NKV = 4
HD = 256
WIN = 1024
ROPE_BASE = 10000.0
EPS = 1e-6
SCALING = HD ** -0.5

NCORES = 8
CA = 512            # tokens per projection chunk (phase A)
NCA = S // CA       # 8
NKC = NCA // 2      # 4 local kv chunks
CB = 256            # queries per attention block (phase B)
NCB = S // CB       # 16
KT = HID // 128     # 20 hid k-tiles
W0, W1 = 3, 10      # w/h startup split points: kt 0-2, 3-9, 10-19
f32 = mybir.dt.float32
f32r = mybir.dt.float32r
bf16 = mybir.dt.bfloat16
AF = mybir.ActivationFunctionType
BF = ml_dtypes.bfloat16

_NC = None
_last_results = None


def _install_ntff_shim():
    """antenv.axon_hooks is absent in this image; rebuild it over libaxon so
    run_bass_kernel_spmd(trace=True) can capture NTFF profiles."""
    if "antenv.axon_hooks" in sys.modules:
        return
    so_path = "/opt/axon/libaxon_pjrt.so"
    hook = None
    try:
        lib = ctypes.CDLL(so_path)
        if hasattr(lib, "axon_start_nrt_profile"):
            lib.axon_start_nrt_profile.argtypes = [
                ctypes.POINTER(ctypes.c_int64),
                ctypes.c_size_t,
            ]
            lib.axon_start_nrt_profile.restype = ctypes.c_int64
            lib.axon_stop_nrt_profile.argtypes = [ctypes.c_char_p]
            lib.axon_stop_nrt_profile.restype = ctypes.c_int64

            @contextlib.contextmanager
            def _hook(output_dir, device_ids):
                import jax

                jax.devices()
                if device_ids:
                    ids = (ctypes.c_int64 * len(device_ids))(*device_ids)
                    rc = lib.axon_start_nrt_profile(ids, len(device_ids))
                else:
                    rc = lib.axon_start_nrt_profile(None, 0)
                if rc != 0:
                    raise RuntimeError(f"axon_start_nrt_profile rc={rc}")
                try:
                    yield
                finally:
                    n = lib.axon_stop_nrt_profile(str(output_dir).encode())
                    if n < 0:
                        raise RuntimeError(f"axon_stop_nrt_profile rc={n}")

            hook = _hook
    except OSError:
        pass
    mod = types.ModuleType("antenv.axon_hooks")
    mod.get_axon_ntff_profile_hook = lambda: hook
    mod.set_axon_ntff_profile_hook = lambda h: None
    sys.modules["antenv.axon_hooks"] = mod


def _phase_a(tc, nc, hT, hTkv, w, cs, cskv, ow, msk, cs_sb, cskv_sb, ow_sb,
             msk_sb, inw_sb, on_sb, on1_sb, qT, kv, vch, qkvres):
    """A1: k+v projection for this core's HALF of the sequence, with a
    per-chunk pair-AllGather through a DRAM bounce; unpack DMAs on the
    gpsimd queue right after each collective. A2: q projection for the
    full sequence (overlaps the collectives + unpacks)."""

    def norm_rope(small, rtmp, nrmp, rbp, wo, dest, xps, cos, sin):
        x0p, x1p = xps
        sq0 = sqpool.tile([128, CA], bf16, tag="sq")
        sq1 = sqpool.tile([128, CA], bf16, tag="sq")
        nc.scalar.activation(sq0, x0p, AF.Square, bias=0.0,
                             scale=inw_sb[:, wo:wo + 1])
        nc.scalar.activation(sq1, x1p, AF.Square, bias=0.0,
                             scale=inw_sb[:, wo + 1:wo + 2])
        ssq = nrmp.tile([1, CA], f32, tag="nrm")
        nc.tensor.matmul(ssq, on_sb[:, 0:1], sq0, start=True, stop=False)
        nc.tensor.matmul(ssq, on_sb[:, 0:1], sq1, start=False, stop=True)
        t1 = small.tile([1, CA], f32, tag="t1")
        nc.scalar.activation(t1, ssq, AF.Copy, bias=EPS, scale=1.0 / HD)
        t2 = small.tile([1, CA], f32, tag="t2")
        nc.vector.reciprocal_approx_fast(out=t2, in_=t1)
        rstd = small.tile([1, CA], f32r, tag="rstd")
        nc.scalar.activation(rstd, t2, AF.Sqrt, bias=0.0, scale=1.0)
        rb = rbp.tile([128, CA], f32, tag="rb")
        nc.tensor.matmul(rb, on1_sb, rstd, start=True, stop=True)
        ra = rtmp.tile([128, CA], f32, tag="m")
        rb_ = rtmp.tile([128, CA], f32, tag="m")
        nc.vector.tensor_mul(ra, x0p, cos)
        nc.vector.tensor_mul(rb_, x1p, sin)
        re = rtmp.tile([128, CA], f32, tag="m")
        nc.vector.tensor_sub(re, ra, rb_)
        rc = rtmp.tile([128, CA], f32, tag="m")
        rd = rtmp.tile([128, CA], f32, tag="m")
        nc.vector.tensor_mul(rc, x1p, cos)
        nc.vector.tensor_mul(rd, x0p, sin)
        rf = rtmp.tile([128, CA], f32, tag="m")
        nc.vector.tensor_add(rf, rc, rd)
        nc.vector.tensor_mul(dest[:, 0:CA], re, rb)
        nc.vector.tensor_mul(dest[:, CA:2 * CA], rf, rb)

    with tc.tile_pool(name="hTt", bufs=2) as hpool, \
         tc.tile_pool(name="hT0", bufs=1) as h0pool, \
         tc.tile_pool(name="wt", bufs=1) as wpool, \
         tc.tile_pool(name="kvloc", bufs=2) as kvlpool, \
         tc.tile_pool(name="vloc", bufs=2) as vlpool, \
         tc.tile_pool(name="sq", bufs=2) as sqpool, \
         tc.tile_pool(name="rtmpA", bufs=4) as rtmp, \
         tc.tile_pool(name="smallA", bufs=1) as small, \
         tc.tile_pool(name="dramx", bufs=1, space="DRAM") as dram, \
         tc.tile_pool(name="xp", bufs=4, space="PSUM") as xpp, \
         tc.tile_pool(name="vps", bufs=2, space="PSUM") as vpp, \
         tc.tile_pool(name="nrm", bufs=1, space="PSUM") as nrmp, \
         tc.tile_pool(name="rbp", bufs=1, space="PSUM") as rbp:

        # ---- startup DMAs: 3-way split w (sync then scalar queue) and
        # 3-way split h chunk 0 (sync), all big contiguous transfers ----
        w_a = wpool.tile([128, W0 * 768], bf16, tag="w_a")
        nc.sync.dma_start(out=w_a, in_=w[:, 0:W0 * 768])
        w_b = wpool.tile([128, (W1 - W0) * 768], bf16, tag="w_b")
        nc.scalar.dma_start(out=w_b, in_=w[:, W0 * 768:W1 * 768])
        w_c = wpool.tile([128, (KT - W1) * 768], bf16, tag="w_c")
        nc.scalar.dma_start(out=w_c, in_=w[:, W1 * 768:KT * 768])

        def wk(k):
            if k < W0:
                return w_a[:, k * 768:(k + 1) * 768]
            if k < W1:
                return w_b[:, (k - W0) * 768:(k - W0 + 1) * 768]
            return w_c[:, (k - W1) * 768:(k - W1 + 1) * 768]

        h0 = [h0pool.tile([128, W0 * CA], bf16, tag="h0a", name="h0a"),
              h0pool.tile([128, (W1 - W0) * CA], bf16, tag="h0b", name="h0b"),
              h0pool.tile([128, (KT - W1) * CA], bf16, tag="h0c", name="h0c")]
        nc.sync.dma_start(out=h0[0], in_=hTkv[:, 0:W0 * CA])
        nc.sync.dma_start(out=h0[1], in_=hTkv[:, W0 * CA:W1 * CA])
        nc.sync.dma_start(out=h0[2], in_=hTkv[:, W1 * CA:KT * CA])
        nc.gpsimd.dma_start(out=cskv_sb, in_=cskv)
        # bulky later-phase constants on the scalar queue, after w_b/w_c:
        # cs needed at first A2 rope (~75us), msk/ow only in phase B.
        nc.scalar.dma_start(out=cs_sb, in_=cs)
        nc.scalar.dma_start(out=msk_sb, in_=msk)
        nc.scalar.dma_start(out=ow_sb, in_=ow)

        bounce_in = [dram.tile([128, 2048], bf16, name=f"bin{a}")
                     for a in range(NKC)]
        bounce_out = [dram.tile([256, 2048], bf16, name=f"bout{a}")
                      for a in range(NKC)]

        # ---- A1: k+v for the local half-sequence, per-chunk collective ----
        for a in range(NKC):
            if a == 0:
                hslc = [h0[0][:, k * CA:(k + 1) * CA] for k in range(W0)] + \
                       [h0[1][:, (k - W0) * CA:(k - W0 + 1) * CA]
                        for k in range(W0, W1)] + \
                       [h0[2][:, (k - W1) * CA:(k - W1 + 1) * CA]
                        for k in range(W1, KT)]
            else:
                hTt = hpool.tile([128, KT * CA], bf16, tag="hTt")
                nc.sync.dma_start(out=hTt,
                                  in_=hTkv[:, a * KT * CA:(a + 1) * KT * CA])
                hslc = [hTt[:, k * CA:(k + 1) * CA] for k in range(KT)]
            cos = cskv_sb[:, a * 2 * CA: a * 2 * CA + CA]
            sin = cskv_sb[:, a * 2 * CA + CA: (a + 1) * 2 * CA]

            kvt = kvlpool.tile([128, 2 * CA], bf16, tag="kvloc")
            vloc = vlpool.tile([128, 4 * HD], bf16, tag="vloc")
            x_k = [xpp.tile([128, CA], f32, tag="xp", name=f"xk{a}_{j}")
                   for j in range(2)]
            vp = [vpp.tile([128, 2 * HD], f32, tag="vps", name=f"vp{a}_{j}")
                  for j in range(2)]
            for k in range(KT):
                st_, sp = (k == 0), (k == KT - 1)
                wt = wk(k)
                for j in range(2):
                    nc.tensor.matmul(
                        x_k[j], wt[:, 256 + j * 128:256 + (j + 1) * 128],
                        hslc[k], start=st_, stop=sp)
                for st in range(CA // 128):
                    # st%2==1 shares the bank with st%2==0: the opener's
                    # bank-wide has_written clear covers it, so its first
                    # matmul must NOT re-clear (start=False, overwrite via
                    # cleared bits).
                    nc.tensor.matmul(
                        vp[st // 2][:, (st % 2) * HD:(st % 2 + 1) * HD],
                        hslc[k][:, st * 128:(st + 1) * 128],
                        wt[:, 512:768], start=st_ and st % 2 == 0, stop=sp,
                        skip_group_check=st % 2 == 1)

            # v copies first on the scalar queue so the vps banks free
            # before the next chunk's v matmuls need them.
            nc.scalar.activation(vloc[:, 0:2 * HD], vp[0], AF.Copy,
                                 bias=0.0, scale=1.0)
            nc.scalar.activation(vloc[:, 2 * HD:4 * HD], vp[1], AF.Copy,
                                 bias=0.0, scale=1.0)
            norm_rope(small, rtmp, nrmp, rbp, 2, kvt, x_k, cos, sin)

            nc.gpsimd.dma_start(out=bounce_in[a][:, 0:1024], in_=kvt)
            nc.gpsimd.dma_start(out=bounce_in[a][:, 1024:2048], in_=vloc)
            nc.gpsimd.collective_compute(
                "AllGather",
                mybir.AluOpType.bypass,
                replica_groups=[[0, 1], [2, 3], [4, 5], [6, 7]],
                ins=[bounce_in[a].opt()],
                outs=[bounce_out[a].opt()],
            )
            # unpack both pair-halves on the gpsimd queue; chunk a of the
            # even core lands in rows 0:128, chunk NKC+a of the odd core
            # in rows 128:256 (global convention, same on every core).
            for half in range(2):
                g = half * NKC + a
                kvg = qkvres.tile([128, 2 * CA], bf16, tag=f"kv{g}")
                nc.gpsimd.dma_start(
                    out=kvg,
                    in_=bounce_out[a][half * 128:half * 128 + 128, 0:1024])
                kv[g] = kvg
                vcg = qkvres.tile([128, 4 * HD], bf16, tag=f"v{g}")
                nc.gpsimd.dma_start(
                    out=vcg,
                    in_=bounce_out[a][half * 128:half * 128 + 128, 1024:2048])
                vch[g] = vcg

        # ---- A2: q for the full sequence (overlaps collectives) ----
        for g in range(NCA):
            hTt = hpool.tile([128, KT * CA], bf16, tag="hTt")
            nc.sync.dma_start(out=hTt,
                              in_=hT[:, g * KT * CA:(g + 1) * KT * CA])
            hslc = [hTt[:, k * CA:(k + 1) * CA] for k in range(KT)]
            cos = cs_sb[:, g * 2 * CA: g * 2 * CA + CA]
            sin = cs_sb[:, g * 2 * CA + CA: (g + 1) * 2 * CA]

            qTt = qkvres.tile([128, 2 * CA], bf16, tag=f"qT{g}")
            qT[g] = qTt
            x_q = [xpp.tile([128, CA], f32, tag="xp", name=f"xq{g}_{j}")
                   for j in range(2)]
            for k in range(KT):
                st_, sp = (k == 0), (k == KT - 1)
                wt = wk(k)
                for j in range(2):
                    nc.tensor.matmul(
                        x_q[j], wt[:, j * 128:(j + 1) * 128],
                        hslc[k], start=st_, stop=sp)
            norm_rope(small, rtmp, nrmp, rbp, 0, qTt, x_q, cos, sin)


def _phase_b(tc, nc, ow_sb, msk_sb, on_sb, on1_sb, qT, kv, vch, outp):
    with tc.tile_pool(name="probs", bufs=12) as ppool, \
         tc.tile_pool(name="attnT", bufs=4) as apool, \
         tc.tile_pool(name="osb", bufs=2) as opool, \
         tc.tile_pool(name="ibsp", bufs=2) as ipool, \
         tc.tile_pool(name="smallB", bufs=2) as small, \
         tc.tile_pool(name="sc", bufs=3, space="PSUM") as scp, \
         tc.tile_pool(name="pv", bufs=2, space="PSUM") as pvp, \
         tc.tile_pool(name="sums", bufs=1, space="PSUM") as smp, \
         tc.tile_pool(name="op", bufs=2, space="PSUM") as opp:

        def attn(t):
            a, half = t // 2, t % 2
            qs = qT[a]
            # k-subtiles, full-width ones first (they open the accumulation
            # groups); the two half-masked edges compute only the valid
            # 128-query half.  (kt, qoff, width, mask)
            plan = []
            for kt in range(max(0, 2 * t - 7), 2 * t):
                m = ("edge", 128) if kt == 2 * t - 7 else None
                plan.append((kt, 0, CB, m))
            plan.append((2 * t, 0, CB, ("diag", 0)))
            if 2 * t - 8 >= 0:
                plan.append((2 * t - 8, 0, 128, ("edge", 0)))
            plan.append((2 * t + 1, 128, 128, ("diag", 128)))

            prs = []
            for kt, qoff, width, maskspec in plan:
                ca, sb = kt // 4, kt % 4
                kvsrc = kv[ca]
                sc = scp.tile([128, CB], f32, tag="sc")
                scv = sc[:, qoff:qoff + width]
                for h in range(2):
                    nc.tensor.matmul(
                        scv,
                        kvsrc[:, h * CA + sb * 128: h * CA + sb * 128 + 128],
                        qs[:, h * CA + half * CB + qoff:
                           h * CA + half * CB + qoff + width],
                        start=(h == 0), stop=(h == 1))
                pr = ppool.tile([128, CB], bf16, tag="pr")
                prv = pr[:, qoff:qoff + width]
                nc.scalar.activation(prv, scv, AF.Exp, bias=0.0,
                                     scale=SCALING)
                if maskspec is not None:
                    kind, moff = maskspec
                    m = msk_sb[:, 0:128] if kind == "edge" \
                        else msk_sb[:, 256:384]
                    nc.vector.tensor_mul(pr[:, moff:moff + 128],
                                         pr[:, moff:moff + 128], m)
                prs.append(prv)

            sums = smp.tile([1, CB], f32, tag="sums")
            for i, ((kt, qoff, width, _), prv) in enumerate(zip(plan, prs)):
                nc.tensor.matmul(sums[:, qoff:qoff + width], on_sb[:, 0:1],
                                 prv, start=(i == 0), stop=(i == len(prs) - 1))
            pv = pvp.tile([128, 2 * CB], f32, tag="pv")
            pv0, pv1 = pv[:, 0:CB], pv[:, CB:2 * CB]
            for i, ((kt, qoff, width, _), prv) in enumerate(zip(plan, prs)):
                first, last = (i == 0), (i == len(plan) - 1)
                v_ = vch[kt // 4][:, (kt % 4) * HD:(kt % 4 + 1) * HD]
                # pv1 shares the bank with pv0: only pv0's first matmul
                # opens (bank-wide clear); pv1's first overwrites via the
                # cleared has_written bits (start=False).
                nc.tensor.matmul(pv0[:, qoff:qoff + width], v_[:, 0:128], prv,
                                 start=first, stop=last)
                nc.tensor.matmul(pv1[:, qoff:qoff + width], v_[:, 128:256],
                                 prv, start=False, stop=last,
                                 skip_group_check=True)

            sc_ = small.tile([1, CB], f32r, tag="sc_")
            nc.scalar.activation(sc_, sums, AF.Copy, bias=0.0, scale=1.0)
            rbv = scp.tile([128, CB], f32, tag="sc")
            nc.tensor.matmul(rbv, on1_sb, sc_, start=True, stop=True)
            ibs = ipool.tile([128, CB], f32, tag="ibs")
            nc.vector.reciprocal_approx_fast(out=ibs, in_=rbv)
            at0 = apool.tile([128, CB], bf16, tag="at")
            at1 = apool.tile([128, CB], bf16, tag="at")
            nc.vector.tensor_mul(at0, pv0, ibs)
            nc.vector.tensor_mul(at1, pv1, ibs)
            return at0, at1

        def oproj(t, at0, at1):
            t0 = t * CB
            for st in range(2):
                ob = opool.tile([128, HID], bf16, tag="ob")
                for hc in range(HID // 512):
                    op = opp.tile([128, 512], f32, tag="op")
                    nc.tensor.matmul(op, at0[:, st * 128:(st + 1) * 128],
                                     ow_sb[:, hc * 512:(hc + 1) * 512],
                                     start=True, stop=False)
                    nc.tensor.matmul(op, at1[:, st * 128:(st + 1) * 128],
                                     ow_sb[:, HID + hc * 512:
                                           HID + (hc + 1) * 512],
                                     start=False, stop=True)
                    if hc in (0, 2):
                        nc.vector.tensor_copy(ob[:, hc * 512:(hc + 1) * 512],
                                              op)
                    else:
                        nc.scalar.activation(ob[:, hc * 512:(hc + 1) * 512],
                                             op, AF.Copy, bias=0.0, scale=1.0)
                    if hc == 1:
                        nc.sync.dma_start(
                            out=outp[t0 + st * 128:t0 + (st + 1) * 128,
                                     0:1024],
                            in_=ob[:, 0:1024])
                nc.sync.dma_start(
                    out=outp[t0 + st * 128:t0 + (st + 1) * 128, 1024:HID],
                    in_=ob[:, 1024:HID])

        # software pipeline: o_proj of block t-1 sits behind the attention
        # matmuls of block t, hiding the softmax-denominator chain.
        prev = None
        for t in range(NCB):
            at0, at1 = attn(t)
            if prev is not None:
                oproj(prev[0], prev[1], prev[2])
            prev = (t, at0, at1)
        oproj(prev[0], prev[1], prev[2])


def _body(ctx, tc, hT, hTkv, w, ow, cs, cskv, msk, inw, on, on1, outp):
    nc = tc.nc

    const = ctx.enter_context(tc.tile_pool(name="const", bufs=1))
    qkvres = ctx.enter_context(tc.tile_pool(name="qkvres", bufs=1))

    inw_sb = const.tile([128, 4], f32)
    nc.sync.dma_start(out=inw_sb, in_=inw)
    on_sb = const.tile([128, 2], bf16)
    nc.sync.dma_start(out=on_sb, in_=on)
    on1_sb = const.tile([1, 128], f32r)
    nc.sync.dma_start(out=on1_sb, in_=on1)
    cs_sb = const.tile([128, NCA * 2 * CA], bf16)
    cskv_sb = const.tile([128, NCA * CA], bf16)
    ow_sb = const.tile([128, 2 * HID], bf16)
    msk_sb = const.tile([128, 384], bf16)

    qT = {}
    kv = {}
    vch = {}

    _phase_a(tc, nc, hT, hTkv, w, cs, cskv, ow, msk, cs_sb, cskv_sb, ow_sb,
             msk_sb, inw_sb, on_sb, on1_sb, qT, kv, vch, qkvres)
    _phase_b(tc, nc, ow_sb, msk_sb, on_sb, on1_sb, qT, kv, vch, outp)


def _build():
    nc = bacc.Bacc("TRN2", target_bir_lowering=False, debug=False,
                   num_devices=NCORES)
    hT = nc.dram_tensor("hT", [128, KT * S], bf16, kind="ExternalInput").ap()
    hTkv = nc.dram_tensor("hTkv", [128, KT * S // 2], bf16,
                          kind="ExternalInput").ap()
    cskv = nc.dram_tensor("cskv", [128, NCA * CA], bf16,
                          kind="ExternalInput").ap()
    w = nc.dram_tensor("w", [128, KT * 768], bf16, kind="ExternalInput").ap()
    ow = nc.dram_tensor("ow", [128, 2 * HID], bf16, kind="ExternalInput").ap()
    cs = nc.dram_tensor("cs", [128, NCA * 2 * CA], bf16, kind="ExternalInput").ap()
    msk = nc.dram_tensor("msk", [128, 384], bf16, kind="ExternalInput").ap()
    inw = nc.dram_tensor("inw", [128, 4], f32, kind="ExternalInput").ap()
    on = nc.dram_tensor("on", [128, 2], bf16, kind="ExternalInput").ap()
    on1 = nc.dram_tensor("on1", [1, 128], f32r, kind="ExternalInput").ap()
    outp = nc.dram_tensor("outp", [S, HID], bf16, kind="ExternalOutput").ap()
    with tile.TileContext(nc) as tc, ExitStack() as ctx:
        with nc.allow_low_precision(reason="bf16 matmul pipeline"):
            _body(ctx, tc, hT, hTkv, w, ow, cs, cskv, msk, inw, on, on1, outp)
    nc.compile()
    return nc


def _get_nc():
    global _NC
    if _NC is None:
        _NC = _build()
    return _NC


def build_in_maps(positions, hidden_states, qkv_w, o_w, q_norm_w, k_norm_w):
    positions = np.asarray(positions)
    hidden_states = np.asarray(hidden_states, dtype=np.float32)
    qkv_w = np.asarray(qkv_w, dtype=np.float32)
    o_w = np.asarray(o_w, dtype=np.float32)
    q_norm_w = np.asarray(q_norm_w, dtype=np.float32)
    k_norm_w = np.asarray(k_norm_w, dtype=np.float32)
    assert np.array_equal(positions.astype(np.int64), np.arange(S)), \
        "kernel assumes contiguous arange positions (banded sliding window)"

    hT0 = hidden_states.T  # [HID, S]
    hT = np.ascontiguousarray(
        hT0.reshape(KT, 128, NCA, CA).transpose(1, 2, 0, 3)
        .reshape(128, KT * S)).astype(BF)

    inv_freq = 1.0 / (ROPE_BASE ** (np.arange(0, HD, 2, dtype=np.float32) / HD))
    freqs = positions.astype(np.float32)[:, None] * inv_freq[None, :]  # [S,128]
    cos_t = np.ascontiguousarray(np.cos(freqs).T.astype(np.float32))
    sin_t = np.ascontiguousarray(np.sin(freqs).T.astype(np.float32))
    csb = np.stack([cos_t.reshape(128, NCA, CA), sin_t.reshape(128, NCA, CA)],
                   axis=2)  # [128, NCA, 2, CA]
    cs = np.ascontiguousarray(csb.reshape(128, NCA * 2 * CA)).astype(BF)

    kl = np.arange(128)[:, None]
    ql = np.arange(128)[None, :]
    edge = (kl > ql).astype(np.float32)
    diag = (kl <= ql).astype(np.float32)
    zero = np.zeros((128, 128), np.float32)
    msk = np.concatenate([edge, zero, diag], axis=1).astype(BF)  # [128, 384]

    nwq = 1.0 + q_norm_w
    nwk = 1.0 + k_norm_w
    inw = np.stack([1.0 / nwq[:128], 1.0 / nwq[128:],
                    1.0 / nwk[:128], 1.0 / nwk[128:]], axis=1)
    inw = np.ascontiguousarray(inw.astype(np.float32))  # [128, 4]

    on = np.ones((128, 2), BF)
    on1 = np.ones((1, 128), np.float32)

    in_maps = []
    for c in range(NCORES):
        g = c // 2
        wq = qkv_w[:, c * HD:(c + 1) * HD] * nwq[None, :]
        wk = qkv_w[:, NH * HD + g * HD:NH * HD + (g + 1) * HD] * nwk[None, :]
        wv = qkv_w[:, (NH + NKV) * HD + g * HD:(NH + NKV) * HD + (g + 1) * HD]
        wslice = np.concatenate([wq, wk, wv], axis=1).astype(np.float32)
        wslice = np.ascontiguousarray(
            wslice.reshape(KT, 128, 768).transpose(1, 0, 2)
            .reshape(128, KT * 768)).astype(BF)
        owslice = o_w[c * HD:(c + 1) * HD, :].astype(np.float32)
        owslice = np.ascontiguousarray(
            owslice.reshape(2, 128, HID).transpose(1, 0, 2)
            .reshape(128, 2 * HID)).astype(BF)
        halfsz = KT * S // 2
        hTkv = np.ascontiguousarray(hT[:, (c % 2) * halfsz:
                                        (c % 2 + 1) * halfsz])
        cskv = np.ascontiguousarray(cs[:, (c % 2) * (NCA * CA):
                                       (c % 2 + 1) * (NCA * CA)])
        in_maps.append({
            "hT": hT, "hTkv": hTkv, "cskv": cskv, "w": wslice, "ow": owslice,
            "cs": cs, "msk": msk, "inw": inw, "on": on, "on1": on1,
        })
    return in_maps


def kernel(positions, hidden_states, qkv_w, o_w, q_norm_w, k_norm_w):
    global _last_results
    _install_ntff_shim()
    in_maps = build_in_maps(positions, hidden_states, qkv_w, o_w,
                            q_norm_w, k_norm_w)

    nc = _get_nc()
    res = run_bass_kernel_spmd(nc, in_maps, list(range(NCORES)))
    _last_results = res

    out = res.results[0]["outp"].astype(np.float32)
    for c in range(1, NCORES):
        out = out + res.results[c]["outp"].astype(np.float32)
    return out


# revision 11
# speedup vs baseline: 1.1267x; 1.0717x over previous
"""Gemma3 sliding-window attention layer on 8 Trainium2 NeuronCores.

Sharding: tensor-parallel over heads. Core c computes q-head c and kv-head c//2
(kv heads duplicated across the 2 cores sharing them), then the o_proj
row-slice for its head. The 8 partial outputs are summed on the host.

v4 (vs v3):
- startup: 3-way split contiguous w/h DMAs (big transfers at high BW)
  instead of 40 small per-kt DMAs -> first matmul ~5us instead of ~24us.
- per-chunk pair-AllGather (4 small collectives issued as each local kv
  chunk completes) instead of 2 late ones -> collective fully hidden
  under the q-projection pass; unpack DMAs ride the gpsimd queue right
  after each collective so phase B never waits.
- shared 3-buf h pool gates A2 prefetch behind A1 compute (no early
  bandwidth steal), single [128,10240] DMA per chunk.
- phase B software pipeline: o_proj of block t-1 emitted after the
  attention matmuls of block t, so the softmax-denominator chain
  (sums -> sc_ -> rbv -> ibs -> at) never stalls the PE.
- PSUM pools rebalanced: A = xp4+vps2+ssq1+rb1, B = sc3+pv2+sums1+op2
  (pv0|pv1 packed per-bank; rbv allocated from the sc pool).
- v tiles packed per chunk [128, 4*HD]; 2 bounce writes + 4 unpacks per
  chunk instead of 5/5.
"""
import os
import sys
import types
import contextlib
import ctypes

import numpy as np
import ml_dtypes

for _p in ("/opt/trn_rl_repo", "/root/.axon_site/_ro/trn_rl_repo"):
    if os.path.isdir(_p) and _p not in sys.path:
        sys.path.insert(0, _p)

from contextlib import ExitStack

import concourse.bass as bass
import concourse.mybir as mybir
import concourse.tile as tile
from concourse import bacc
from concourse.bass_utils import run_bass_kernel_spmd

S = 4096
HID = 2560
NH = 8
NKV = 4
HD = 256
WIN = 1024
ROPE_BASE = 10000.0
EPS = 1e-6
SCALING = HD ** -0.5

NCORES = 8
CA = 512            # tokens per projection chunk (phase A)
NCA = S // CA       # 8
NKC = NCA // 2      # 4 local kv chunks
CB = 256            # queries per attention block (phase B)
NCB = S // CB       # 16
KT = HID // 128     # 20 hid k-tiles
W0, W1 = 3, 10      # w/h startup split points: kt 0-2, 3-9, 10-19
f32 = mybir.dt.float32
f32r = mybir.dt.float32r
bf16 = mybir.dt.bfloat16
AF = mybir.ActivationFunctionType
BF = ml_dtypes.bfloat16

_NC = None
_last_results = None


def _install_ntff_shim():
    """antenv.axon_hooks is absent in this image; rebuild it over libaxon so
    run_bass_kernel_spmd(trace=True) can capture NTFF profiles."""
    if "antenv.axon_hooks" in sys.modules:
        return
    so_path = "/opt/axon/libaxon_pjrt.so"
    hook = None
    try:
        lib = ctypes.CDLL(so_path)
        if hasattr(lib, "axon_start_nrt_profile"):
            lib.axon_start_nrt_profile.argtypes = [
                ctypes.POINTER(ctypes.c_int64),
                ctypes.c_size_t,
            ]
            lib.axon_start_nrt_profile.restype = ctypes.c_int64
            lib.axon_stop_nrt_profile.argtypes = [ctypes.c_char_p]
            lib.axon_stop_nrt_profile.restype = ctypes.c_int64

            @contextlib.contextmanager
            def _hook(output_dir, device_ids):
                import jax

                jax.devices()
                if device_ids:
                    ids = (ctypes.c_int64 * len(device_ids))(*device_ids)
                    rc = lib.axon_start_nrt_profile(ids, len(device_ids))
                else:
                    rc = lib.axon_start_nrt_profile(None, 0)
                if rc != 0:
                    raise RuntimeError(f"axon_start_nrt_profile rc={rc}")
                try:
                    yield
                finally:
                    n = lib.axon_stop_nrt_profile(str(output_dir).encode())
                    if n < 0:
                        raise RuntimeError(f"axon_stop_nrt_profile rc={n}")

            hook = _hook
    except OSError:
        pass
    mod = types.ModuleType("antenv.axon_hooks")
    mod.get_axon_ntff_profile_hook = lambda: hook
    mod.set_axon_ntff_profile_hook = lambda h: None
    sys.modules["antenv.axon_hooks"] = mod


def _phase_a(tc, nc, hT, hTkv, w, cs, cskv, ow, msk, cs_sb, cskv_sb, ow_sb,
             msk_sb, inw_sb, on_sb, on1_sb, qT, kv, vch, qkvres):
    """A1: k+v projection for this core's HALF of the sequence, with a
    per-chunk pair-AllGather through a DRAM bounce; unpack DMAs on the
    gpsimd queue right after each collective. A2: q projection for the
    full sequence (overlaps the collectives + unpacks)."""

    def norm_rope(small, rtmp, nrmp, rbp, wo, dest, xps, cos, sin):
        x0p, x1p = xps
        sq0 = sqpool.tile([128, CA], bf16, tag="sq")
        sq1 = sqpool.tile([128, CA], bf16, tag="sq")
        nc.scalar.activation(sq0, x0p, AF.Square, bias=0.0,
                             scale=inw_sb[:, wo:wo + 1])
        nc.scalar.activation(sq1, x1p, AF.Square, bias=0.0,
                             scale=inw_sb[:, wo + 1:wo + 2])
        ssq = nrmp.tile([1, CA], f32, tag="nrm")
        nc.tensor.matmul(ssq, on_sb[:, 0:1], sq0, start=True, stop=False)
        nc.tensor.matmul(ssq, on_sb[:, 0:1], sq1, start=False, stop=True)
        t1 = small.tile([1, CA], f32, tag="t1")
        nc.scalar.activation(t1, ssq, AF.Copy, bias=EPS, scale=1.0 / HD)
        t2 = small.tile([1, CA], f32, tag="t2")
        nc.vector.reciprocal_approx_fast(out=t2, in_=t1)
        rstd = small.tile([1, CA], f32r, tag="rstd")
        nc.scalar.activation(rstd, t2, AF.Sqrt, bias=0.0, scale=1.0)
        rb = rbp.tile([128, CA], f32, tag="rb")
        nc.tensor.matmul(rb, on1_sb, rstd, start=True, stop=True)
        ra = rtmp.tile([128, CA], f32, tag="m")
        rb_ = rtmp.tile([128, CA], f32, tag="m")
        nc.vector.tensor_mul(ra, x0p, cos)
        nc.vector.tensor_mul(rb_, x1p, sin)
        re = rtmp.tile([128, CA], f32, tag="m")
        nc.vector.tensor_sub(re, ra, rb_)
        rc = rtmp.tile([128, CA], f32, tag="m")
        rd = rtmp.tile([128, CA], f32, tag="m")
        nc.vector.tensor_mul(rc, x1p, cos)
        nc.vector.tensor_mul(rd, x0p, sin)
        rf = rtmp.tile([128, CA], f32, tag="m")
        nc.vector.tensor_add(rf, rc, rd)
        nc.vector.tensor_mul(dest[:, 0:CA], re, rb)
        nc.vector.tensor_mul(dest[:, CA:2 * CA], rf, rb)

    with tc.tile_pool(name="hTt", bufs=10) as hpool, \
         tc.tile_pool(name="wt", bufs=1) as wpool, \
         tc.tile_pool(name="kvloc", bufs=2) as kvlpool, \
         tc.tile_pool(name="vloc", bufs=2) as vlpool, \
         tc.tile_pool(name="sq", bufs=2) as sqpool, \
         tc.tile_pool(name="rtmpA", bufs=4) as rtmp, \
         tc.tile_pool(name="smallA", bufs=1) as small, \
         tc.tile_pool(name="dramx", bufs=1, space="DRAM") as dram, \
         tc.tile_pool(name="xp", bufs=4, space="PSUM") as xpp, \
         tc.tile_pool(name="vps", bufs=2, space="PSUM") as vpp, \
         tc.tile_pool(name="nrm", bufs=1, space="PSUM") as nrmp, \
         tc.tile_pool(name="rbp", bufs=1, space="PSUM") as rbp:

        # ---- startup DMAs in strict need order on ONE queue (sync) so
        # the critical chain (w kt0-2, h quarter 0, ...) is never starved
        # by round-robin with bulk transfers. Bulky later-phase constants
        # (cs/msk/ow) are emitted inside the A1 loop body so their issue
        # is gated behind early compute on the scalar queue.
        QH = 5                       # h quarter = 5 k-tiles
        w_a = wpool.tile([128, W0 * 768], bf16, tag="w_a")
        nc.sync.dma_start(out=w_a, in_=w[:, 0:W0 * 768])

        def load_quarter(src, a, q):
            t = hpool.tile([128, QH * CA], bf16, tag="hTt")
            base = (a * KT + q * QH) * CA
            nc.sync.dma_start(out=t, in_=src[:, base:base + QH * CA])
            return t

        hq0 = [load_quarter(hTkv, 0, 0)]
        w_b = wpool.tile([128, (W1 - W0) * 768], bf16, tag="w_b")
        nc.sync.dma_start(out=w_b, in_=w[:, W0 * 768:W1 * 768])
        hq0.append(load_quarter(hTkv, 0, 1))
        w_c = wpool.tile([128, (KT - W1) * 768], bf16, tag="w_c")
        nc.sync.dma_start(out=w_c, in_=w[:, W1 * 768:KT * 768])
        hq0.append(load_quarter(hTkv, 0, 2))
        hq0.append(load_quarter(hTkv, 0, 3))
        nc.gpsimd.dma_start(out=cskv_sb, in_=cskv)

        def wk(k):
            if k < W0:
                return w_a[:, k * 768:(k + 1) * 768]
            if k < W1:
                return w_b[:, (k - W0) * 768:(k - W0 + 1) * 768]
            return w_c[:, (k - W1) * 768:(k - W1 + 1) * 768]

        bounce_in = [dram.tile([128, 2048], bf16, name=f"bin{a}")
                     for a in range(NKC)]
        bounce_out = [dram.tile([256, 2048], bf16, name=f"bout{a}")
                      for a in range(NKC)]

        # ---- A1: k+v for the local half-sequence, per-chunk collective ----
        for a in range(NKC):
            hq = hq0 if a == 0 else [load_quarter(hTkv, a, q)
                                     for q in range(KT // QH)]
            hslc = [hq[k // QH][:, (k % QH) * CA:(k % QH + 1) * CA]
                    for k in range(KT)]
            if a == 1:
                nc.scalar.dma_start(out=cs_sb, in_=cs)
            elif a == 2:
                nc.scalar.dma_start(out=msk_sb, in_=msk)
                nc.scalar.dma_start(out=ow_sb, in_=ow)
            cos = cskv_sb[:, a * 2 * CA: a * 2 * CA + CA]
            sin = cskv_sb[:, a * 2 * CA + CA: (a + 1) * 2 * CA]

            kvt = kvlpool.tile([128, 2 * CA], bf16, tag="kvloc")
            vloc = vlpool.tile([128, 4 * HD], bf16, tag="vloc")
            x_k = [xpp.tile([128, CA], f32, tag="xp", name=f"xk{a}_{j}")
                   for j in range(2)]
            vp = [vpp.tile([128, 2 * HD], f32, tag="vps", name=f"vp{a}_{j}")
                  for j in range(2)]
            for k in range(KT):
                st_, sp = (k == 0), (k == KT - 1)
                wt = wk(k)
                for j in range(2):
                    nc.tensor.matmul(
                        x_k[j], wt[:, 256 + j * 128:256 + (j + 1) * 128],
                        hslc[k], start=st_, stop=sp)
                for st in range(CA // 128):
                    # st%2==1 shares the bank with st%2==0: the opener's
                    # bank-wide has_written clear covers it, so its first
                    # matmul must NOT re-clear (start=False, overwrite via
                    # cleared bits).
                    nc.tensor.matmul(
                        vp[st // 2][:, (st % 2) * HD:(st % 2 + 1) * HD],
                        hslc[k][:, st * 128:(st + 1) * 128],
                        wt[:, 512:768], start=st_ and st % 2 == 0, stop=sp,
                        skip_group_check=st % 2 == 1)

            # v copies first on the scalar queue so the vps banks free
            # before the next chunk's v matmuls need them.
            nc.scalar.activation(vloc[:, 0:2 * HD], vp[0], AF.Copy,
                                 bias=0.0, scale=1.0)
            nc.scalar.activation(vloc[:, 2 * HD:4 * HD], vp[1], AF.Copy,
                                 bias=0.0, scale=1.0)
            norm_rope(small, rtmp, nrmp, rbp, 2, kvt, x_k, cos, sin)

            nc.gpsimd.dma_start(out=bounce_in[a][:, 0:1024], in_=kvt)
            nc.gpsimd.dma_start(out=bounce_in[a][:, 1024:2048], in_=vloc)
            nc.gpsimd.collective_compute(
                "AllGather",
                mybir.AluOpType.bypass,
                replica_groups=[[0, 1], [2, 3], [4, 5], [6, 7]],
                ins=[bounce_in[a].opt()],
                outs=[bounce_out[a].opt()],
            )
            # unpack both pair-halves on the gpsimd queue; chunk a of the
            # even core lands in rows 0:128, chunk NKC+a of the odd core
            # in rows 128:256 (global convention, same on every core).
            for half in range(2):
                g = half * NKC + a
                kvg = qkvres.tile([128, 2 * CA], bf16, tag=f"kv{g}")
                nc.gpsimd.dma_start(
                    out=kvg,
                    in_=bounce_out[a][half * 128:half * 128 + 128, 0:1024])
                kv[g] = kvg
                vcg = qkvres.tile([128, 4 * HD], bf16, tag=f"v{g}")
                nc.gpsimd.dma_start(
                    out=vcg,
                    in_=bounce_out[a][half * 128:half * 128 + 128, 1024:2048])
                vch[g] = vcg

        # ---- A2: q for the full sequence (overlaps collectives) ----
        for g in range(NCA):
            hq = [load_quarter(hT, g, q) for q in range(KT // QH)]
            hslc = [hq[k // QH][:, (k % QH) * CA:(k % QH + 1) * CA]
                    for k in range(KT)]
            cos = cs_sb[:, g * 2 * CA: g * 2 * CA + CA]
            sin = cs_sb[:, g * 2 * CA + CA: (g + 1) * 2 * CA]

            qTt = qkvres.tile([128, 2 * CA], bf16, tag=f"qT{g}")
            qT[g] = qTt
            x_q = [xpp.tile([128, CA], f32, tag="xp", name=f"xq{g}_{j}")
                   for j in range(2)]
            for k in range(KT):
                st_, sp = (k == 0), (k == KT - 1)
                wt = wk(k)
                for j in range(2):
                    nc.tensor.matmul(
                        x_q[j], wt[:, j * 128:(j + 1) * 128],
                        hslc[k], start=st_, stop=sp)
            norm_rope(small, rtmp, nrmp, rbp, 0, qTt, x_q, cos, sin)


def _phase_b(tc, nc, ow_sb, msk_sb, on_sb, on1_sb, qT, kv, vch, outp):
    with tc.tile_pool(name="probs", bufs=12) as ppool, \
         tc.tile_pool(name="attnT", bufs=4) as apool, \
         tc.tile_pool(name="osb", bufs=2) as opool, \
         tc.tile_pool(name="ibsp", bufs=2) as ipool, \
         tc.tile_pool(name="smallB", bufs=2) as small, \
         tc.tile_pool(name="sc", bufs=3, space="PSUM") as scp, \
         tc.tile_pool(name="pv", bufs=2, space="PSUM") as pvp, \
         tc.tile_pool(name="sums", bufs=1, space="PSUM") as smp, \
         tc.tile_pool(name="op", bufs=2, space="PSUM") as opp:

        def attn(t):
            a, half = t // 2, t % 2
            qs = qT[a]
            # k-subtiles, full-width ones first (they open the accumulation
            # groups); the two half-masked edges compute only the valid
            # 128-query half.  (kt, qoff, width, mask)
            plan = []
            for kt in range(max(0, 2 * t - 7), 2 * t):
                m = ("edge", 128) if kt == 2 * t - 7 else None
                plan.append((kt, 0, CB, m))
            plan.append((2 * t, 0, CB, ("diag", 0)))
            if 2 * t - 8 >= 0:
                plan.append((2 * t - 8, 0, 128, ("edge", 0)))
            plan.append((2 * t + 1, 128, 128, ("diag", 128)))

            prs = []
            for kt, qoff, width, maskspec in plan:
                ca, sb = kt // 4, kt % 4
                kvsrc = kv[ca]
                sc = scp.tile([128, CB], f32, tag="sc")
                scv = sc[:, qoff:qoff + width]
                for h in range(2):
                    nc.tensor.matmul(
                        scv,
                        kvsrc[:, h * CA + sb * 128: h * CA + sb * 128 + 128],
                        qs[:, h * CA + half * CB + qoff:
                           h * CA + half * CB + qoff + width],
                        start=(h == 0), stop=(h == 1))
                pr = ppool.tile([128, CB], bf16, tag="pr")
                prv = pr[:, qoff:qoff + width]
                nc.scalar.activation(prv, scv, AF.Exp, bias=0.0,
                                     scale=SCALING)
                if maskspec is not None:
                    kind, moff = maskspec
                    m = msk_sb[:, 0:128] if kind == "edge" \
                        else msk_sb[:, 256:384]
                    nc.vector.tensor_mul(pr[:, moff:moff + 128],
                                         pr[:, moff:moff + 128], m)
                prs.append(prv)

            sums = smp.tile([1, CB], f32, tag="sums")
            for i, ((kt, qoff, width, _), prv) in enumerate(zip(plan, prs)):
                nc.tensor.matmul(sums[:, qoff:qoff + width], on_sb[:, 0:1],
                                 prv, start=(i == 0), stop=(i == len(prs) - 1))
            pv = pvp.tile([128, 2 * CB], f32, tag="pv")
            pv0, pv1 = pv[:, 0:CB], pv[:, CB:2 * CB]
            for i, ((kt, qoff, width, _), prv) in enumerate(zip(plan, prs)):
                first, last = (i == 0), (i == len(plan) - 1)
                v_ = vch[kt // 4][:, (kt % 4) * HD:(kt % 4 + 1) * HD]
                # pv1 shares the bank with pv0: only pv0's first matmul
                # opens (bank-wide clear); pv1's first overwrites via the
                # cleared has_written bits (start=False).
                nc.tensor.matmul(pv0[:, qoff:qoff + width], v_[:, 0:128], prv,
                                 start=first, stop=last)
                nc.tensor.matmul(pv1[:, qoff:qoff + width], v_[:, 128:256],
                                 prv, start=False, stop=last,
                                 skip_group_check=True)

            sc_ = small.tile([1, CB], f32r, tag="sc_")
            nc.scalar.activation(sc_, sums, AF.Copy, bias=0.0, scale=1.0)
            rbv = scp.tile([128, CB], f32, tag="sc")
            nc.tensor.matmul(rbv, on1_sb, sc_, start=True, stop=True)
            ibs = ipool.tile([128, CB], f32, tag="ibs")
            nc.vector.reciprocal_approx_fast(out=ibs, in_=rbv)
            at0 = apool.tile([128, CB], bf16, tag="at")
            at1 = apool.tile([128, CB], bf16, tag="at")
            nc.vector.tensor_mul(at0, pv0, ibs)
            nc.vector.tensor_mul(at1, pv1, ibs)
            return at0, at1

        def oproj(t, at0, at1):
            t0 = t * CB
            for st in range(2):
                ob = opool.tile([128, HID], bf16, tag="ob")
                for hc in range(HID // 512):
                    op = opp.tile([128, 512], f32, tag="op")
                    nc.tensor.matmul(op, at0[:, st * 128:(st + 1) * 128],
                                     ow_sb[:, hc * 512:(hc + 1) * 512],
                                     start=True, stop=False)
                    nc.tensor.matmul(op, at1[:, st * 128:(st + 1) * 128],
                                     ow_sb[:, HID + hc * 512:
                                           HID + (hc + 1) * 512],
                                     start=False, stop=True)
                    if hc in (0, 2, 4):
                        nc.vector.tensor_copy(ob[:, hc * 512:(hc + 1) * 512],
                                              op)
                    else:
                        nc.scalar.activation(ob[:, hc * 512:(hc + 1) * 512],
                                             op, AF.Copy, bias=0.0, scale=1.0)
                    if hc == 1:
                        nc.sync.dma_start(
                            out=outp[t0 + st * 128:t0 + (st + 1) * 128,
                                     0:1024],
                            in_=ob[:, 0:1024])
                nc.sync.dma_start(
                    out=outp[t0 + st * 128:t0 + (st + 1) * 128, 1024:HID],
                    in_=ob[:, 1024:HID])

        # software pipeline: o_proj of block t-1 sits behind the attention
        # matmuls of block t, hiding the softmax-denominator chain.
        prev = None
        for t in range(NCB):
            at0, at1 = attn(t)
            if prev is not None:
                oproj(prev[0], prev[1], prev[2])
            prev = (t, at0, at1)
        oproj(prev[0], prev[1], prev[2])


def _body(ctx, tc, hT, hTkv, w, ow, cs, cskv, msk, inw, on, on1, outp):
    nc = tc.nc

    const = ctx.enter_context(tc.tile_pool(name="const", bufs=1))
    qkvres = ctx.enter_context(tc.tile_pool(name="qkvres", bufs=1))

    inw_sb = const.tile([128, 4], f32)
    nc.sync.dma_start(out=inw_sb, in_=inw)
    on_sb = const.tile([128, 2], bf16)
    nc.sync.dma_start(out=on_sb, in_=on)
    on1_sb = const.tile([1, 128], f32r)
    nc.sync.dma_start(out=on1_sb, in_=on1)
    cs_sb = const.tile([128, NCA * 2 * CA], bf16)
    cskv_sb = const.tile([128, NCA * CA], bf16)
    ow_sb = const.tile([128, 2 * HID], bf16)
    msk_sb = const.tile([128, 384], bf16)

    qT = {}
    kv = {}
    vch = {}

    _phase_a(tc, nc, hT, hTkv, w, cs, cskv, ow, msk, cs_sb, cskv_sb, ow_sb,
             msk_sb, inw_sb, on_sb, on1_sb, qT, kv, vch, qkvres)
    _phase_b(tc, nc, ow_sb, msk_sb, on_sb, on1_sb, qT, kv, vch, outp)


def _build():
    nc = bacc.Bacc("TRN2", target_bir_lowering=False, debug=False,
                   num_devices=NCORES)
    hT = nc.dram_tensor("hT", [128, KT * S], bf16, kind="ExternalInput").ap()
    hTkv = nc.dram_tensor("hTkv", [128, KT * S // 2], bf16,
                          kind="ExternalInput").ap()
    cskv = nc.dram_tensor("cskv", [128, NCA * CA], bf16,
                          kind="ExternalInput").ap()
    w = nc.dram_tensor("w", [128, KT * 768], bf16, kind="ExternalInput").ap()
    ow = nc.dram_tensor("ow", [128, 2 * HID], bf16, kind="ExternalInput").ap()
    cs = nc.dram_tensor("cs", [128, NCA * 2 * CA], bf16, kind="ExternalInput").ap()
    msk = nc.dram_tensor("msk", [128, 384], bf16, kind="ExternalInput").ap()
    inw = nc.dram_tensor("inw", [128, 4], f32, kind="ExternalInput").ap()
    on = nc.dram_tensor("on", [128, 2], bf16, kind="ExternalInput").ap()
    on1 = nc.dram_tensor("on1", [1, 128], f32r, kind="ExternalInput").ap()
    outp = nc.dram_tensor("outp", [S, HID], bf16, kind="ExternalOutput").ap()
    with tile.TileContext(nc) as tc, ExitStack() as ctx:
        with nc.allow_low_precision(reason="bf16 matmul pipeline"):
            _body(ctx, tc, hT, hTkv, w, ow, cs, cskv, msk, inw, on, on1, outp)
    nc.compile()
    return nc


def _get_nc():
    global _NC
    if _NC is None:
        _NC = _build()
    return _NC


def build_in_maps(positions, hidden_states, qkv_w, o_w, q_norm_w, k_norm_w):
    positions = np.asarray(positions)
    hidden_states = np.asarray(hidden_states, dtype=np.float32)
    qkv_w = np.asarray(qkv_w, dtype=np.float32)
    o_w = np.asarray(o_w, dtype=np.float32)
    q_norm_w = np.asarray(q_norm_w, dtype=np.float32)
    k_norm_w = np.asarray(k_norm_w, dtype=np.float32)
    assert np.array_equal(positions.astype(np.int64), np.arange(S)), \
        "kernel assumes contiguous arange positions (banded sliding window)"

    hT0 = hidden_states.T  # [HID, S]
    hT = np.ascontiguousarray(
        hT0.reshape(KT, 128, NCA, CA).transpose(1, 2, 0, 3)
        .reshape(128, KT * S)).astype(BF)

    inv_freq = 1.0 / (ROPE_BASE ** (np.arange(0, HD, 2, dtype=np.float32) / HD))
    freqs = positions.astype(np.float32)[:, None] * inv_freq[None, :]  # [S,128]
    cos_t = np.ascontiguousarray(np.cos(freqs).T.astype(np.float32))
    sin_t = np.ascontiguousarray(np.sin(freqs).T.astype(np.float32))
    csb = np.stack([cos_t.reshape(128, NCA, CA), sin_t.reshape(128, NCA, CA)],
                   axis=2)  # [128, NCA, 2, CA]
    cs = np.ascontiguousarray(csb.reshape(128, NCA * 2 * CA)).astype(BF)

    kl = np.arange(128)[:, None]
    ql = np.arange(128)[None, :]
    edge = (kl > ql).astype(np.float32)
    diag = (kl <= ql).astype(np.float32)
    zero = np.zeros((128, 128), np.float32)
    msk = np.concatenate([edge, zero, diag], axis=1).astype(BF)  # [128, 384]

    nwq = 1.0 + q_norm_w
    nwk = 1.0 + k_norm_w
    inw = np.stack([1.0 / nwq[:128], 1.0 / nwq[128:],
                    1.0 / nwk[:128], 1.0 / nwk[128:]], axis=1)
    inw = np.ascontiguousarray(inw.astype(np.float32))  # [128, 4]

    on = np.ones((128, 2), BF)
    on1 = np.ones((1, 128), np.float32)

    in_maps = []
    for c in range(NCORES):
        g = c // 2
        wq = qkv_w[:, c * HD:(c + 1) * HD] * nwq[None, :]
        wk = qkv_w[:, NH * HD + g * HD:NH * HD + (g + 1) * HD] * nwk[None, :]
        wv = qkv_w[:, (NH + NKV) * HD + g * HD:(NH + NKV) * HD + (g + 1) * HD]
        wslice = np.concatenate([wq, wk, wv], axis=1).astype(np.float32)
        wslice = np.ascontiguousarray(
            wslice.reshape(KT, 128, 768).transpose(1, 0, 2)
            .reshape(128, KT * 768)).astype(BF)
        owslice = o_w[c * HD:(c + 1) * HD, :].astype(np.float32)
        owslice = np.ascontiguousarray(
            owslice.reshape(2, 128, HID).transpose(1, 0, 2)
            .reshape(128, 2 * HID)).astype(BF)
        halfsz = KT * S // 2
        hTkv = np.ascontiguousarray(hT[:, (c % 2) * halfsz:
                                        (c % 2 + 1) * halfsz])
        cskv = np.ascontiguousarray(cs[:, (c % 2) * (NCA * CA):
                                       (c % 2 + 1) * (NCA * CA)])
        in_maps.append({
            "hT": hT, "hTkv": hTkv, "cskv": cskv, "w": wslice, "ow": owslice,
            "cs": cs, "msk": msk, "inw": inw, "on": on, "on1": on1,
        })
    return in_maps


def kernel(positions, hidden_states, qkv_w, o_w, q_norm_w, k_norm_w):
    global _last_results
    _install_ntff_shim()
    in_maps = build_in_maps(positions, hidden_states, qkv_w, o_w,
                            q_norm_w, k_norm_w)

    nc = _get_nc()
    res = run_bass_kernel_spmd(nc, in_maps, list(range(NCORES)))
    _last_results = res

    out = res.results[0]["outp"].astype(np.float32)
    for c in range(1, NCORES):
        out = out + res.results[c]["outp"].astype(np.float32)
    return out


# revision 15
# speedup vs baseline: 1.1625x; 1.0318x over previous
"""Gemma3 sliding-window attention layer on 8 Trainium2 NeuronCores.

Sharding: tensor-parallel over heads. Core c computes q-head c and kv-head c//2
(kv heads duplicated across the 2 cores sharing them), then the o_proj
row-slice for its head. The 8 partial outputs are summed on the host.

v4 (vs v3):
- startup: 3-way split contiguous w/h DMAs (big transfers at high BW)
  instead of 40 small per-kt DMAs -> first matmul ~5us instead of ~24us.
- per-chunk pair-AllGather (4 small collectives issued as each local kv
  chunk completes) instead of 2 late ones -> collective fully hidden
  under the q-projection pass; unpack DMAs ride the gpsimd queue right
  after each collective so phase B never waits.
- shared 3-buf h pool gates A2 prefetch behind A1 compute (no early
  bandwidth steal), single [128,10240] DMA per chunk.
- phase B software pipeline: o_proj of block t-1 emitted after the
  attention matmuls of block t, so the softmax-denominator chain
  (sums -> sc_ -> rbv -> ibs -> at) never stalls the PE.
- PSUM pools rebalanced: A = xp4+vps2+ssq1+rb1, B = sc3+pv2+sums1+op2
  (pv0|pv1 packed per-bank; rbv allocated from the sc pool).
- v tiles packed per chunk [128, 4*HD]; 2 bounce writes + 4 unpacks per
  chunk instead of 5/5.
"""
import os
import sys
import types
import contextlib
import ctypes

import numpy as np
import ml_dtypes

for _p in ("/opt/trn_rl_repo", "/root/.axon_site/_ro/trn_rl_repo"):
    if os.path.isdir(_p) and _p not in sys.path:
        sys.path.insert(0, _p)

from contextlib import ExitStack

import concourse.bass as bass
import concourse.mybir as mybir
import concourse.tile as tile
from concourse import bacc
from concourse.bass_utils import run_bass_kernel_spmd

S = 4096
HID = 2560
NH = 8
NKV = 4
HD = 256
WIN = 1024
ROPE_BASE = 10000.0
EPS = 1e-6
SCALING = HD ** -0.5

NCORES = 8
CA = 512            # tokens per projection chunk (phase A)
NCA = S // CA       # 8
NKC = NCA // 2      # 4 local kv chunks
CB = 256            # queries per attention block (phase B)
NCB = S // CB       # 16
KT = HID // 128     # 20 hid k-tiles
W0, W1 = 3, 10      # w/h startup split points: kt 0-2, 3-9, 10-19
f32 = mybir.dt.float32
f32r = mybir.dt.float32r
bf16 = mybir.dt.bfloat16
AF = mybir.ActivationFunctionType
BF = ml_dtypes.bfloat16

_NC = None
_last_results = None


def _install_ntff_shim():
    """antenv.axon_hooks is absent in this image; rebuild it over libaxon so
    run_bass_kernel_spmd(trace=True) can capture NTFF profiles."""
    if "antenv.axon_hooks" in sys.modules:
        return
    so_path = "/opt/axon/libaxon_pjrt.so"
    hook = None
    try:
        lib = ctypes.CDLL(so_path)
        if hasattr(lib, "axon_start_nrt_profile"):
            lib.axon_start_nrt_profile.argtypes = [
                ctypes.POINTER(ctypes.c_int64),
                ctypes.c_size_t,
            ]
            lib.axon_start_nrt_profile.restype = ctypes.c_int64
            lib.axon_stop_nrt_profile.argtypes = [ctypes.c_char_p]
            lib.axon_stop_nrt_profile.restype = ctypes.c_int64

            @contextlib.contextmanager
            def _hook(output_dir, device_ids):
                import jax

                jax.devices()
                if device_ids:
                    ids = (ctypes.c_int64 * len(device_ids))(*device_ids)
                    rc = lib.axon_start_nrt_profile(ids, len(device_ids))
                else:
                    rc = lib.axon_start_nrt_profile(None, 0)
                if rc != 0:
                    raise RuntimeError(f"axon_start_nrt_profile rc={rc}")
                try:
                    yield
                finally:
                    n = lib.axon_stop_nrt_profile(str(output_dir).encode())
                    if n < 0:
                        raise RuntimeError(f"axon_stop_nrt_profile rc={n}")

            hook = _hook
    except OSError:
        pass
    mod = types.ModuleType("antenv.axon_hooks")
    mod.get_axon_ntff_profile_hook = lambda: hook
    mod.set_axon_ntff_profile_hook = lambda h: None
    sys.modules["antenv.axon_hooks"] = mod


def _phase_a(tc, nc, hT, hTkv, w, cs, cskv, ow, msk, cs_sb, cskv_sb, ow_sb,
             msk_sb, inw_sb, on_sb, on1_sb, qT, kv, vch, qkvres):
    """A1: k+v projection for this core's HALF of the sequence, with a
    per-chunk pair-AllGather through a DRAM bounce; unpack DMAs on the
    gpsimd queue right after each collective. A2: q projection for the
    full sequence (overlaps the collectives + unpacks)."""

    def norm_rope(small, rtmp, nrmp, rbp, wo, dest, xps, cos, sin):
        x0p, x1p = xps
        sq0 = sqpool.tile([128, CA], bf16, tag="sq")
        sq1 = sqpool.tile([128, CA], bf16, tag="sq")
        nc.scalar.activation(sq0, x0p, AF.Square, bias=0.0,
                             scale=inw_sb[:, wo:wo + 1])
        nc.scalar.activation(sq1, x1p, AF.Square, bias=0.0,
                             scale=inw_sb[:, wo + 1:wo + 2])
        ssq = nrmp.tile([1, CA], f32, tag="nrm")
        nc.tensor.matmul(ssq, on_sb[:, 0:1], sq0, start=True, stop=False)
        nc.tensor.matmul(ssq, on_sb[:, 0:1], sq1, start=False, stop=True)
        t1 = small.tile([1, CA], f32, tag="t1")
        nc.scalar.activation(t1, ssq, AF.Copy, bias=EPS, scale=1.0 / HD)
        t2 = small.tile([1, CA], f32, tag="t2")
        nc.vector.reciprocal_approx_fast(out=t2, in_=t1)
        rstd = small.tile([1, CA], f32r, tag="rstd")
        nc.scalar.activation(rstd, t2, AF.Sqrt, bias=0.0, scale=1.0)
        rb = rbp.tile([128, CA], f32, tag="rb")
        nc.tensor.matmul(rb, on1_sb, rstd, start=True, stop=True)
        ra = rtmp.tile([128, CA], f32, tag="m")
        rb_ = rtmp.tile([128, CA], f32, tag="m")
        nc.vector.tensor_mul(ra, x0p, cos)
        nc.vector.tensor_mul(rb_, x1p, sin)
        re = rtmp.tile([128, CA], f32, tag="m")
        nc.vector.tensor_sub(re, ra, rb_)
        rc = rtmp.tile([128, CA], f32, tag="m")
        rd = rtmp.tile([128, CA], f32, tag="m")
        nc.vector.tensor_mul(rc, x1p, cos)
        nc.vector.tensor_mul(rd, x0p, sin)
        rf = rtmp.tile([128, CA], f32, tag="m")
        nc.vector.tensor_add(rf, rc, rd)
        nc.vector.tensor_mul(dest[:, 0:CA], re, rb)
        nc.vector.tensor_mul(dest[:, CA:2 * CA], rf, rb)

    with tc.tile_pool(name="hTt", bufs=10) as hpool, \
         tc.tile_pool(name="wt", bufs=1) as wpool, \
         tc.tile_pool(name="kvloc", bufs=2) as kvlpool, \
         tc.tile_pool(name="vloc", bufs=2) as vlpool, \
         tc.tile_pool(name="sq", bufs=2) as sqpool, \
         tc.tile_pool(name="rtmpA", bufs=4) as rtmp, \
         tc.tile_pool(name="smallA", bufs=1) as small, \
         tc.tile_pool(name="dramx", bufs=1, space="DRAM") as dram, \
         tc.tile_pool(name="xp", bufs=4, space="PSUM") as xpp, \
         tc.tile_pool(name="vps", bufs=2, space="PSUM") as vpp, \
         tc.tile_pool(name="nrm", bufs=1, space="PSUM") as nrmp, \
         tc.tile_pool(name="rbp", bufs=1, space="PSUM") as rbp:

        # ---- startup DMAs in strict need order on ONE queue (sync) so
        # the critical chain (w kt0-2, h quarter 0, ...) is never starved
        # by round-robin with bulk transfers. Bulky later-phase constants
        # (cs/msk/ow) are emitted inside the A1 loop body so their issue
        # is gated behind early compute on the scalar queue.
        QH = 5                       # h quarter = 5 k-tiles
        w_a = wpool.tile([128, W0 * 768], bf16, tag="w_a")
        nc.sync.dma_start(out=w_a, in_=w[:, 0:W0 * 768])

        def load_quarter(src, a, q):
            t = hpool.tile([128, QH * CA], bf16, tag="hTt")
            base = (a * KT + q * QH) * CA
            nc.sync.dma_start(out=t, in_=src[:, base:base + QH * CA])
            return t

        hq0 = [load_quarter(hTkv, 0, 0)]
        w_b = wpool.tile([128, (W1 - W0) * 768], bf16, tag="w_b")
        nc.sync.dma_start(out=w_b, in_=w[:, W0 * 768:W1 * 768])
        hq0.append(load_quarter(hTkv, 0, 1))
        w_c = wpool.tile([128, (KT - W1) * 768], bf16, tag="w_c")
        nc.sync.dma_start(out=w_c, in_=w[:, W1 * 768:KT * 768])
        hq0.append(load_quarter(hTkv, 0, 2))
        hq0.append(load_quarter(hTkv, 0, 3))
        nc.gpsimd.dma_start(out=cskv_sb, in_=cskv)

        def wk(k):
            if k < W0:
                return w_a[:, k * 768:(k + 1) * 768]
            if k < W1:
                return w_b[:, (k - W0) * 768:(k - W0 + 1) * 768]
            return w_c[:, (k - W1) * 768:(k - W1 + 1) * 768]

        bounce_in = [dram.tile([128, 2048], bf16, name=f"bin{a}")
                     for a in range(NKC)]
        bounce_out = [dram.tile([256, 2048], bf16, name=f"bout{a}")
                      for a in range(NKC)]

        # ---- A1: k+v for the local half-sequence, per-chunk collective ----
        for a in range(NKC):
            hq = hq0 if a == 0 else [load_quarter(hTkv, a, q)
                                     for q in range(KT // QH)]
            hslc = [hq[k // QH][:, (k % QH) * CA:(k % QH + 1) * CA]
                    for k in range(KT)]
            cos = cskv_sb[:, a * 2 * CA: a * 2 * CA + CA]
            sin = cskv_sb[:, a * 2 * CA + CA: (a + 1) * 2 * CA]

            kvt = kvlpool.tile([128, 2 * CA], bf16, tag="kvloc")
            vloc = vlpool.tile([128, 4 * HD], bf16, tag="vloc")
            x_k = [xpp.tile([128, CA], f32, tag="xp", name=f"xk{a}_{j}")
                   for j in range(2)]
            vp = [vpp.tile([128, 2 * HD], f32, tag="vps", name=f"vp{a}_{j}")
                  for j in range(2)]
            for k in range(KT):
                st_, sp = (k == 0), (k == KT - 1)
                wt = wk(k)
                for j in range(2):
                    nc.tensor.matmul(
                        x_k[j], wt[:, 256 + j * 128:256 + (j + 1) * 128],
                        hslc[k], start=st_, stop=sp)
                for st in range(CA // 128):
                    # st%2==1 shares the bank with st%2==0: the opener's
                    # bank-wide has_written clear covers it, so its first
                    # matmul must NOT re-clear (start=False, overwrite via
                    # cleared bits).
                    nc.tensor.matmul(
                        vp[st // 2][:, (st % 2) * HD:(st % 2 + 1) * HD],
                        hslc[k][:, st * 128:(st + 1) * 128],
                        wt[:, 512:768], start=st_ and st % 2 == 0, stop=sp,
                        skip_group_check=st % 2 == 1)

            # v copies first on the scalar queue so the vps banks free
            # before the next chunk's v matmuls need them.
            nc.scalar.activation(vloc[:, 0:2 * HD], vp[0], AF.Copy,
                                 bias=0.0, scale=1.0)
            nc.scalar.activation(vloc[:, 2 * HD:4 * HD], vp[1], AF.Copy,
                                 bias=0.0, scale=1.0)
            norm_rope(small, rtmp, nrmp, rbp, 2, kvt, x_k, cos, sin)

            nc.gpsimd.dma_start(out=bounce_in[a][:, 0:1024], in_=kvt)
            nc.gpsimd.dma_start(out=bounce_in[a][:, 1024:2048], in_=vloc)
            nc.gpsimd.collective_compute(
                "AllGather",
                mybir.AluOpType.bypass,
                replica_groups=[[0, 1], [2, 3], [4, 5], [6, 7]],
                ins=[bounce_in[a].opt()],
                outs=[bounce_out[a].opt()],
            )
            if a == 0:
                # bulky later-phase constants: a WAR hazard on a compute
                # product (kvt) keeps the scheduler from hoisting these
                # DMAs into the critical startup window.
                nc.vector.tensor_copy(cs_sb[:, 0:2], kvt[:, 0:2])
                nc.scalar.dma_start(out=cs_sb, in_=cs)
            elif a == 1:
                nc.vector.tensor_copy(msk_sb[:, 0:2], kvt[:, 0:2])
                nc.scalar.dma_start(out=msk_sb, in_=msk)
                nc.vector.tensor_copy(ow_sb[:, 0:2], kvt[:, 0:2])
                nc.scalar.dma_start(out=ow_sb, in_=ow)

        # unpack AFTER all bounce writes + triggers so a waiting unpack
        # never head-of-line-blocks the next chunk's bounce writes on the
        # gpsimd FIFO. Chunk a of the even core lands in rows 0:128,
        # chunk NKC+a of the odd core in rows 128:256 (global convention,
        # identical on every core).
        for a in range(NKC):
            for half in range(2):
                g = half * NKC + a
                kvg = qkvres.tile([128, 2 * CA], bf16, tag=f"kv{g}")
                nc.gpsimd.dma_start(
                    out=kvg,
                    in_=bounce_out[a][half * 128:half * 128 + 128, 0:1024])
                kv[g] = kvg
                vcg = qkvres.tile([128, 4 * HD], bf16, tag=f"v{g}")
                nc.gpsimd.dma_start(
                    out=vcg,
                    in_=bounce_out[a][half * 128:half * 128 + 128, 1024:2048])
                vch[g] = vcg

        # ---- A2: q for the full sequence (overlaps collectives) ----
        for g in range(NCA):
            hq = [load_quarter(hT, g, q) for q in range(KT // QH)]
            hslc = [hq[k // QH][:, (k % QH) * CA:(k % QH + 1) * CA]
                    for k in range(KT)]
            cos = cs_sb[:, g * 2 * CA: g * 2 * CA + CA]
            sin = cs_sb[:, g * 2 * CA + CA: (g + 1) * 2 * CA]

            qTt = qkvres.tile([128, 2 * CA], bf16, tag=f"qT{g}")
            qT[g] = qTt
            x_q = [xpp.tile([128, CA], f32, tag="xp", name=f"xq{g}_{j}")
                   for j in range(2)]
            for k in range(KT):
                st_, sp = (k == 0), (k == KT - 1)
                wt = wk(k)
                for j in range(2):
                    nc.tensor.matmul(
                        x_q[j], wt[:, j * 128:(j + 1) * 128],
                        hslc[k], start=st_, stop=sp)
            norm_rope(small, rtmp, nrmp, rbp, 0, qTt, x_q, cos, sin)


def _phase_b(tc, nc, ow_sb, msk_sb, on_sb, on1_sb, qT, kv, vch, outp):
    with tc.tile_pool(name="probs", bufs=12) as ppool, \
         tc.tile_pool(name="attnT", bufs=4) as apool, \
         tc.tile_pool(name="osb", bufs=2) as opool, \
         tc.tile_pool(name="ibsp", bufs=2) as ipool, \
         tc.tile_pool(name="smallB", bufs=2) as small, \
         tc.tile_pool(name="sc", bufs=3, space="PSUM") as scp, \
         tc.tile_pool(name="pv", bufs=2, space="PSUM") as pvp, \
         tc.tile_pool(name="sums", bufs=1, space="PSUM") as smp, \
         tc.tile_pool(name="op", bufs=2, space="PSUM") as opp:

        def attn(t):
            a, half = t // 2, t % 2
            qs = qT[a]
            # k-subtiles, full-width ones first (they open the accumulation
            # groups); the two half-masked edges compute only the valid
            # 128-query half.  (kt, qoff, width, mask)
            plan = []
            for kt in range(max(0, 2 * t - 7), 2 * t):
                m = ("edge", 128) if kt == 2 * t - 7 else None
                plan.append((kt, 0, CB, m))
            plan.append((2 * t, 0, CB, ("diag", 0)))
            if 2 * t - 8 >= 0:
                plan.append((2 * t - 8, 0, 128, ("edge", 0)))
            plan.append((2 * t + 1, 128, 128, ("diag", 128)))

            prs = []
            for kt, qoff, width, maskspec in plan:
                ca, sb = kt // 4, kt % 4
                kvsrc = kv[ca]
                sc = scp.tile([128, CB], f32, tag="sc")
                scv = sc[:, qoff:qoff + width]
                for h in range(2):
                    nc.tensor.matmul(
                        scv,
                        kvsrc[:, h * CA + sb * 128: h * CA + sb * 128 + 128],
                        qs[:, h * CA + half * CB + qoff:
                           h * CA + half * CB + qoff + width],
                        start=(h == 0), stop=(h == 1))
                pr = ppool.tile([128, CB], bf16, tag="pr")
                prv = pr[:, qoff:qoff + width]
                nc.scalar.activation(prv, scv, AF.Exp, bias=0.0,
                                     scale=SCALING)
                if maskspec is not None:
                    kind, moff = maskspec
                    m = msk_sb[:, 0:128] if kind == "edge" \
                        else msk_sb[:, 256:384]
                    nc.vector.tensor_mul(pr[:, moff:moff + 128],
                                         pr[:, moff:moff + 128], m)
                prs.append(prv)

            sums = smp.tile([1, CB], f32, tag="sums")
            for i, ((kt, qoff, width, _), prv) in enumerate(zip(plan, prs)):
                nc.tensor.matmul(sums[:, qoff:qoff + width], on_sb[:, 0:1],
                                 prv, start=(i == 0), stop=(i == len(prs) - 1))
            pv = pvp.tile([128, 2 * CB], f32, tag="pv")
            pv0, pv1 = pv[:, 0:CB], pv[:, CB:2 * CB]
            for i, ((kt, qoff, width, _), prv) in enumerate(zip(plan, prs)):
                first, last = (i == 0), (i == len(plan) - 1)
                v_ = vch[kt // 4][:, (kt % 4) * HD:(kt % 4 + 1) * HD]
                # pv1 shares the bank with pv0: only pv0's first matmul
                # opens (bank-wide clear); pv1's first overwrites via the
                # cleared has_written bits (start=False).
                nc.tensor.matmul(pv0[:, qoff:qoff + width], v_[:, 0:128], prv,
                                 start=first, stop=last)
                nc.tensor.matmul(pv1[:, qoff:qoff + width], v_[:, 128:256],
                                 prv, start=False, stop=last,
                                 skip_group_check=True)

            sc_ = small.tile([1, CB], f32r, tag="sc_")
            nc.scalar.activation(sc_, sums, AF.Copy, bias=0.0, scale=1.0)
            rbv = scp.tile([128, CB], f32, tag="sc")
            nc.tensor.matmul(rbv, on1_sb, sc_, start=True, stop=True)
            ibs = ipool.tile([128, CB], f32, tag="ibs")
            nc.vector.reciprocal_approx_fast(out=ibs, in_=rbv)
            at0 = apool.tile([128, CB], bf16, tag="at")
            at1 = apool.tile([128, CB], bf16, tag="at")
            nc.vector.tensor_mul(at0, pv0, ibs)
            nc.vector.tensor_mul(at1, pv1, ibs)
            return at0, at1

        def oproj(t, at0, at1, fine=False):
            t0 = t * CB
            for st in range(2):
                ob = opool.tile([128, HID], bf16, tag="ob")
                for hc in range(HID // 512):
                    op = opp.tile([128, 512], f32, tag="op")
                    nc.tensor.matmul(op, at0[:, st * 128:(st + 1) * 128],
                                     ow_sb[:, hc * 512:(hc + 1) * 512],
                                     start=True, stop=False)
                    nc.tensor.matmul(op, at1[:, st * 128:(st + 1) * 128],
                                     ow_sb[:, HID + hc * 512:
                                           HID + (hc + 1) * 512],
                                     start=False, stop=True)
                    if hc in (0, 2, 4):
                        nc.vector.tensor_copy(ob[:, hc * 512:(hc + 1) * 512],
                                              op)
                    else:
                        nc.scalar.activation(ob[:, hc * 512:(hc + 1) * 512],
                                             op, AF.Copy, bias=0.0, scale=1.0)
                    if fine:
                        nc.sync.dma_start(
                            out=outp[t0 + st * 128:t0 + (st + 1) * 128,
                                     hc * 512:(hc + 1) * 512],
                            in_=ob[:, hc * 512:(hc + 1) * 512])
                    elif hc == 1:
                        nc.sync.dma_start(
                            out=outp[t0 + st * 128:t0 + (st + 1) * 128,
                                     0:1024],
                            in_=ob[:, 0:1024])
                if not fine:
                    nc.sync.dma_start(
                        out=outp[t0 + st * 128:t0 + (st + 1) * 128, 1024:HID],
                        in_=ob[:, 1024:HID])

        # software pipeline: o_proj of block t-1 sits behind the attention
        # matmuls of block t, hiding the softmax-denominator chain.
        prev = None
        for t in range(NCB):
            at0, at1 = attn(t)
            if prev is not None:
                oproj(prev[0], prev[1], prev[2])
            prev = (t, at0, at1)
        oproj(prev[0], prev[1], prev[2], fine=True)


def _body(ctx, tc, hT, hTkv, w, ow, cs, cskv, msk, inw, on, on1, outp):
    nc = tc.nc

    const = ctx.enter_context(tc.tile_pool(name="const", bufs=1))
    qkvres = ctx.enter_context(tc.tile_pool(name="qkvres", bufs=1))

    inw_sb = const.tile([128, 4], f32)
    nc.sync.dma_start(out=inw_sb, in_=inw)
    on_sb = const.tile([128, 2], bf16)
    nc.sync.dma_start(out=on_sb, in_=on)
    on1_sb = const.tile([1, 128], f32r)
    nc.sync.dma_start(out=on1_sb, in_=on1)
    cs_sb = const.tile([128, NCA * 2 * CA], bf16)
    cskv_sb = const.tile([128, NCA * CA], bf16)
    ow_sb = const.tile([128, 2 * HID], bf16)
    msk_sb = const.tile([128, 384], bf16)

    qT = {}
    kv = {}
    vch = {}

    _phase_a(tc, nc, hT, hTkv, w, cs, cskv, ow, msk, cs_sb, cskv_sb, ow_sb,
             msk_sb, inw_sb, on_sb, on1_sb, qT, kv, vch, qkvres)
    _phase_b(tc, nc, ow_sb, msk_sb, on_sb, on1_sb, qT, kv, vch, outp)


def _build():
    nc = bacc.Bacc("TRN2", target_bir_lowering=False, debug=False,
                   num_devices=NCORES)
    hT = nc.dram_tensor("hT", [128, KT * S], bf16, kind="ExternalInput").ap()
    hTkv = nc.dram_tensor("hTkv", [128, KT * S // 2], bf16,
                          kind="ExternalInput").ap()
    cskv = nc.dram_tensor("cskv", [128, NCA * CA], bf16,
                          kind="ExternalInput").ap()
    w = nc.dram_tensor("w", [128, KT * 768], bf16, kind="ExternalInput").ap()
    ow = nc.dram_tensor("ow", [128, 2 * HID], bf16, kind="ExternalInput").ap()
    cs = nc.dram_tensor("cs", [128, NCA * 2 * CA], bf16, kind="ExternalInput").ap()
    msk = nc.dram_tensor("msk", [128, 384], bf16, kind="ExternalInput").ap()
    inw = nc.dram_tensor("inw", [128, 4], f32, kind="ExternalInput").ap()
    on = nc.dram_tensor("on", [128, 2], bf16, kind="ExternalInput").ap()
    on1 = nc.dram_tensor("on1", [1, 128], f32r, kind="ExternalInput").ap()
    outp = nc.dram_tensor("outp", [S, HID], bf16, kind="ExternalOutput").ap()
    with tile.TileContext(nc) as tc, ExitStack() as ctx:
        with nc.allow_low_precision(reason="bf16 matmul pipeline"):
            _body(ctx, tc, hT, hTkv, w, ow, cs, cskv, msk, inw, on, on1, outp)
    nc.compile()
    return nc


def _get_nc():
    global _NC
    if _NC is None:
        _NC = _build()
    return _NC


def build_in_maps(positions, hidden_states, qkv_w, o_w, q_norm_w, k_norm_w):
    positions = np.asarray(positions)
    hidden_states = np.asarray(hidden_states, dtype=np.float32)
    qkv_w = np.asarray(qkv_w, dtype=np.float32)
    o_w = np.asarray(o_w, dtype=np.float32)
    q_norm_w = np.asarray(q_norm_w, dtype=np.float32)
    k_norm_w = np.asarray(k_norm_w, dtype=np.float32)
    assert np.array_equal(positions.astype(np.int64), np.arange(S)), \
        "kernel assumes contiguous arange positions (banded sliding window)"

    hT0 = hidden_states.T  # [HID, S]
    hT = np.ascontiguousarray(
        hT0.reshape(KT, 128, NCA, CA).transpose(1, 2, 0, 3)
        .reshape(128, KT * S)).astype(BF)

    inv_freq = 1.0 / (ROPE_BASE ** (np.arange(0, HD, 2, dtype=np.float32) / HD))
    freqs = positions.astype(np.float32)[:, None] * inv_freq[None, :]  # [S,128]
    cos_t = np.ascontiguousarray(np.cos(freqs).T.astype(np.float32))
    sin_t = np.ascontiguousarray(np.sin(freqs).T.astype(np.float32))
    csb = np.stack([cos_t.reshape(128, NCA, CA), sin_t.reshape(128, NCA, CA)],
                   axis=2)  # [128, NCA, 2, CA]
    cs = np.ascontiguousarray(csb.reshape(128, NCA * 2 * CA)).astype(BF)

    kl = np.arange(128)[:, None]
    ql = np.arange(128)[None, :]
    edge = (kl > ql).astype(np.float32)
    diag = (kl <= ql).astype(np.float32)
    zero = np.zeros((128, 128), np.float32)
    msk = np.concatenate([edge, zero, diag], axis=1).astype(BF)  # [128, 384]

    nwq = 1.0 + q_norm_w
    nwk = 1.0 + k_norm_w
    inw = np.stack([1.0 / nwq[:128], 1.0 / nwq[128:],
                    1.0 / nwk[:128], 1.0 / nwk[128:]], axis=1)
    inw = np.ascontiguousarray(inw.astype(np.float32))  # [128, 4]

    on = np.ones((128, 2), BF)
    on1 = np.ones((1, 128), np.float32)

    in_maps = []
    for c in range(NCORES):
        g = c // 2
        wq = qkv_w[:, c * HD:(c + 1) * HD] * nwq[None, :]
        wk = qkv_w[:, NH * HD + g * HD:NH * HD + (g + 1) * HD] * nwk[None, :]
        wv = qkv_w[:, (NH + NKV) * HD + g * HD:(NH + NKV) * HD + (g + 1) * HD]
        wslice = np.concatenate([wq, wk, wv], axis=1).astype(np.float32)
        wslice = np.ascontiguousarray(
            wslice.reshape(KT, 128, 768).transpose(1, 0, 2)
            .reshape(128, KT * 768)).astype(BF)
        owslice = o_w[c * HD:(c + 1) * HD, :].astype(np.float32)
        owslice = np.ascontiguousarray(
            owslice.reshape(2, 128, HID).transpose(1, 0, 2)
            .reshape(128, 2 * HID)).astype(BF)
        halfsz = KT * S // 2
        hTkv = np.ascontiguousarray(hT[:, (c % 2) * halfsz:
                                        (c % 2 + 1) * halfsz])
        cskv = np.ascontiguousarray(cs[:, (c % 2) * (NCA * CA):
                                       (c % 2 + 1) * (NCA * CA)])
        in_maps.append({
            "hT": hT, "hTkv": hTkv, "cskv": cskv, "w": wslice, "ow": owslice,
            "cs": cs, "msk": msk, "inw": inw, "on": on, "on1": on1,
        })
    return in_maps


def kernel(positions, hidden_states, qkv_w, o_w, q_norm_w, k_norm_w):
    global _last_results
    _install_ntff_shim()
    in_maps = build_in_maps(positions, hidden_states, qkv_w, o_w,
                            q_norm_w, k_norm_w)

    nc = _get_nc()
    res = run_bass_kernel_spmd(nc, in_maps, list(range(NCORES)))
    _last_results = res

    out = res.results[0]["outp"].astype(np.float32)
    for c in range(1, NCORES):
        out = out + res.results[c]["outp"].astype(np.float32)
    return out
